# revision 8
# baseline (speedup 1.0000x reference)
"""Sparse (top-k pruned) multi-head attention on 8 Trainium2 NeuronCores.

Sharding: batch(2) x head-groups(4 heads) -> 8 cores.

Per core (4 heads h=0..3, pairs ct=h//2):
  P: q-proj 1-term fp16; k-proj 3-term fp16 hi/lo (precision carrier);
     v-proj 1-term fp16 -> v_sb [128,16,64].
     Scores operands: kstack_h = [khi;klo] dims stacked, qdup_h = [q;q].
  A (per head): scores S^T = kstack^T qdup (one 128-contraction fp16 mm
     per (kb,qs)); ACT exp (scale=1/8) -> E fp16; AV1-ones matmuls give
     row sums l in one PSUM bank (quadrant rows 0/32/64/96);
     r = recip(l) -> broadcast rb fp16; STT pass E*rb accumulates exact
     normalized colsums cs; gpsimd kth_largest gives the exact top-1843
     boundary midpoint; m_keep -> vk = v*m_keep; AV2 = E @ vk;
     O = AV2 * rb.
  O: out-projection partial (tensor-parallel on d_model), f32 partials.
Host sums per-core partials and adds bo.
"""
import sys
for p in ('/opt/trn_rl_repo', '/opt/pypackages'):
    if p not in sys.path:
        sys.path.insert(0, p)
import numpy as np
from contextlib import ExitStack

import concourse.bass as bass
import concourse.bacc as bacc_mod
import concourse.bass_isa as bass_isa
import concourse.tile as tile
import concourse.mybir as mybir
from concourse import library_config

dt = mybir.dt
F = mybir.ActivationFunctionType
A = mybir.AluOpType
AX = mybir.AxisListType

B, S, DM, H, DK = 2, 2048, 1024, 16, 64
HPC = 4              # heads per core
CPC = HPC * DK       # 256 proj columns per core
KEEP = int(S * 0.9)  # 1843
N_CORES = 8
KT = DM // 128       # 8 contraction tiles for projections
NQ = S // 512        # 4 query chunks
NKB = S // 128       # 16 key tiles
NDROP = S - KEEP     # 205
# kth_largest quantile: k_adj = 204, alpha = 0.5 -> midpoint of
# desc[204]/desc[205] of the negated colsums = exact keep/drop boundary.
QUANT = 1.0 - (NDROP - 0.5) / (S - 1)

_CACHE = {}


def _emit(nc):
    ei = lambda n, s, d: nc.dram_tensor(n, s, d, kind="ExternalInput")
    xq = ei("xq", [DM, S], dt.float16)
    xkh = ei("xkh", [DM, S], dt.float16)
    xkl = ei("xkl", [DM, S], dt.float16)
    xv = ei("xv", [DM, S], dt.float16)
    wq = ei("wq", [DM, CPC], dt.float16)
    wkh = ei("wkh", [DM, CPC], dt.float16)
    wkl = ei("wkl", [DM, CPC], dt.float16)
    wv = ei("wv", [DM, CPC], dt.float16)
    wo = ei("wo", [CPC, DM], dt.float16)
    bqi = ei("bqi", [128, 2], dt.float32)
    bki = ei("bki", [128, 2], dt.float32)
    bvi = ei("bvi", [1, CPC], dt.float16)
    out_part = nc.dram_tensor("out_part", [DM, S], dt.float32, kind="ExternalOutput")

    with tile.TileContext(nc) as tc, ExitStack() as ctx:
        nc.gpsimd.load_library(library_config.attn)

        # ---------- long-lived pools ----------
        cpool = ctx.enter_context(tc.tile_pool(name="const", bufs=1))
        qkpool = ctx.enter_context(tc.tile_pool(name="qk", bufs=1))
        vpool = ctx.enter_context(tc.tile_pool(name="vsb", bufs=1))
        ocpool = ctx.enter_context(tc.tile_pool(name="ocat", bufs=1))

        ones_1x128 = cpool.tile([1, 128], dt.float16, tag="c1")
        nc.vector.memset(ones_1x128[:], 1.0)
        onescol = cpool.tile([128, 1], dt.float16, tag="c2")
        nc.vector.memset(onescol[:], 1.0)
        bq_sb = cpool.tile([128, 2], dt.float32, tag="c4")
        nc.sync.dma_start(bq_sb[:], bqi[:, :])
        bk_sb = cpool.tile([128, 2], dt.float32, tag="c5")
        nc.sync.dma_start(bk_sb[:], bki[:, :])

        qdup = [qkpool.tile([128, S], dt.float16, tag=f"qd{h}", name=f"qd{h}")
                for h in range(HPC)]
        kstack = [qkpool.tile([128, S], dt.float16, tag=f"ks{h}", name=f"ks{h}")
                  for h in range(HPC)]
        v_sb = [vpool.tile([128, NKB, 64], dt.float16, tag=f"vs{h}", name=f"vs{h}")
                for h in range(HPC)]
        ocat = [ocpool.tile([128, S], dt.float16, tag=f"oc{i}", name=f"oc{i}")
                for i in range(2)]

        # ---------------- Phase P: projections ----------------
        with tc.tile_pool(name="pW", bufs=1) as wpool, \
             tc.tile_pool(name="pX", bufs=8) as xpool, \
             tc.tile_pool(name="pT", bufs=2) as tpool, \
             tc.tile_pool(name="pPs", bufs=4, space="PSUM") as pj_ps:

            wq_t = wpool.tile([128, KT * CPC], dt.float16)
            wkh_t = wpool.tile([128, KT * CPC], dt.float16)
            wkl_t = wpool.tile([128, KT * CPC], dt.float16)
            wv_t = wpool.tile([128, KT * CPC], dt.float16)
            for kt in range(KT):
                sl = slice(kt * CPC, (kt + 1) * CPC)
                rows = slice(kt * 128, (kt + 1) * 128)
                nc.sync.dma_start(wq_t[:, sl], wq[rows, :])
                nc.sync.dma_start(wkh_t[:, sl], wkh[rows, :])
                nc.sync.dma_start(wkl_t[:, sl], wkl[rows, :])
                nc.sync.dma_start(wv_t[:, sl], wv[rows, :])
            bv_sb = wpool.tile([1, CPC], dt.float16)
            nc.sync.dma_start(bv_sb[:], bvi[:, :])
            ps_bv = pj_ps.tile([128, 512], dt.float32, tag="pj")
            nc.tensor.matmul(ps_bv[:, 0:CPC], ones_1x128[:], bv_sb[:],
                             start=True, stop=True)
            bv_bc = wpool.tile([128, CPC], dt.float32)
            nc.vector.tensor_copy(bv_bc[:], ps_bv[:, 0:CPC])

            # ---- q projection (1-term) + k projection (3-term hi/lo) ----
            xq_t = []
            xkh_t = []
            xkl_t = []
            for kt in range(KT):
                rows = slice(kt * 128, (kt + 1) * 128)
                t1 = xpool.tile([128, S], dt.float16, tag="xq", name=f"xq{kt}")
                nc.sync.dma_start(t1[:], xq[rows, :])
                xq_t.append(t1)
                t2 = xpool.tile([128, S], dt.float16, tag="xkh", name=f"xkh{kt}")
                nc.sync.dma_start(t2[:], xkh[rows, :])
                xkh_t.append(t2)
                t3 = xpool.tile([128, S], dt.float16, tag="xkl", name=f"xkl{kt}")
                nc.sync.dma_start(t3[:], xkl[rows, :])
                xkl_t.append(t3)
            # ct outer so heads (2*ct, 2*ct+1) complete early and phase A
            # can start while ct=1 / v-proj still run.
            for ct in range(2):
                hA, hB = 2 * ct, 2 * ct + 1
                for nt in range(NQ):
                    qs = slice(nt * 512, (nt + 1) * 512)
                    # q: single term
                    psq = pj_ps.tile([128, 512], dt.float32, tag="pj")
                    for kt in range(KT):
                        wsl = slice(kt * CPC + ct * 128, kt * CPC + ct * 128 + 128)
                        nc.tensor.matmul(psq[:], wq_t[:, wsl], xq_t[kt][:, qs],
                                         start=(kt == 0), stop=(kt == KT - 1))
                    # qdup: rows 0:64 then DMA-duplicate to rows 64:128
                    nc.vector.tensor_scalar(
                        out=qdup[hA][0:64, qs], in0=psq[0:64, :],
                        scalar1=bq_sb[0:64, ct:ct + 1], scalar2=None, op0=A.add)
                    nc.vector.tensor_scalar(
                        out=qdup[hB][0:64, qs], in0=psq[64:128, :],
                        scalar1=bq_sb[64:128, ct:ct + 1], scalar2=None, op0=A.add)
                    nc.sync.dma_start(qdup[hA][64:128, qs], qdup[hA][0:64, qs])
                    nc.sync.dma_start(qdup[hB][64:128, qs], qdup[hB][0:64, qs])

                    # k: 3 terms -> f32 psum
                    psk = pj_ps.tile([128, 512], dt.float32, tag="pj")
                    i_mm = 0
                    for kt in range(KT):
                        wsl = slice(kt * CPC + ct * 128, kt * CPC + ct * 128 + 128)
                        for (wt, xt) in ((wkh_t, xkh_t[kt]), (wkh_t, xkl_t[kt]),
                                         (wkl_t, xkh_t[kt])):
                            nc.tensor.matmul(psk[:], wt[:, wsl], xt[:, qs],
                                             start=(i_mm == 0),
                                             stop=(i_mm == 3 * KT - 1))
                            i_mm += 1
                    # kA: hi aligned rows 0:64; lo via tmp + shift-DMA
                    nc.vector.tensor_scalar(
                        out=kstack[hA][0:64, qs], in0=psk[0:64, :],
                        scalar1=bk_sb[0:64, ct:ct + 1], scalar2=None, op0=A.add)
                    tfA = tpool.tile([128, 512], dt.float32, tag="tf")
                    nc.vector.tensor_scalar(
                        out=tfA[0:64, :], in0=psk[0:64, :],
                        scalar1=bk_sb[0:64, ct:ct + 1], scalar2=None, op0=A.add)
                    tA16 = tpool.tile([128, 512], dt.float16, tag="t16")
                    nc.vector.tensor_tensor(out=tA16[0:64, :], in0=tfA[0:64, :],
                                            in1=kstack[hA][0:64, qs], op=A.subtract)
                    nc.sync.dma_start(kstack[hA][64:128, qs], tA16[0:64, :])
                    # kB: hi into tmp rows 64:128 (aligned), shift-DMA to rows 0:64;
                    #     lo aligned rows 64:128
                    tB16 = tpool.tile([128, 512], dt.float16, tag="t16")
                    nc.vector.tensor_scalar(
                        out=tB16[64:128, :], in0=psk[64:128, :],
                        scalar1=bk_sb[64:128, ct:ct + 1], scalar2=None, op0=A.add)
                    nc.sync.dma_start(kstack[hB][0:64, qs], tB16[64:128, :])
                    tfB = tpool.tile([128, 512], dt.float32, tag="tf")
                    nc.vector.tensor_scalar(
                        out=tfB[64:128, :], in0=psk[64:128, :],
                        scalar1=bk_sb[64:128, ct:ct + 1], scalar2=None, op0=A.add)
                    nc.vector.tensor_tensor(out=kstack[hB][64:128, qs],
                                            in0=tfB[64:128, :], in1=tB16[64:128, :],
                                            op=A.subtract)

            # ---- v projection (1-term fp16) ----
            xv_t = []
            for kt in range(KT):
                rows = slice(kt * 128, (kt + 1) * 128)
                t4 = xpool.tile([128, S], dt.float16, tag="xv", name=f"xv{kt}")
                nc.sync.dma_start(t4[:], xv[rows, :])
                xv_t.append(t4)
            for tb in range(NKB):
                tsl = slice(tb * 128, (tb + 1) * 128)
                psv = pj_ps.tile([128, 512], dt.float32, tag="pj")
                for kt in range(KT):
                    nc.tensor.matmul(psv[:, 0:CPC], xv_t[kt][:, tsl],
                                     wv_t[:, kt * CPC:(kt + 1) * CPC],
                                     start=(kt == 0), stop=(kt == KT - 1))
                for h in range(HPC):
                    nc.vector.tensor_tensor(
                        out=v_sb[h][:, tb, :], in0=psv[:, h * 64:(h + 1) * 64],
                        in1=bv_bc[:, h * 64:(h + 1) * 64], op=A.add)

        # ---------------- Phase A: per-head attention ----------------
        with tc.tile_pool(name="scps", bufs=2, space="PSUM") as score_ps, \
             tc.tile_pool(name="avps", bufs=2, space="PSUM") as av1_ps, \
             tc.tile_pool(name="av2ps", bufs=1, space="PSUM") as av2_ps, \
             tc.tile_pool(name="bcps", bufs=1, space="PSUM") as bc_ps, \
             tc.tile_pool(name="E", bufs=32) as epool, \
             tc.tile_pool(name="rb", bufs=2) as rbpool, \
             tc.tile_pool(name="scr", bufs=1) as scrpool, \
             tc.tile_pool(name="cs", bufs=2) as cspool, \
             tc.tile_pool(name="ls", bufs=1) as lspool, \
             tc.tile_pool(name="vk", bufs=2) as vkpool:

            hctx = {}

            def emit_scores(h):
                e_t = []
                av1 = av1_ps.tile([128, 512], dt.float32, tag="av1",
                                  name=f"av1_{h}")
                for kb in range(NKB):
                    et = epool.tile([128, S], dt.float16, tag="E",
                                    name=f"E{h}_{kb}")
                    e_t.append(et)
                    kcols = slice(kb * 128, (kb + 1) * 128)
                    for half in range(2):
                        sc = score_ps.tile([128, 1024], dt.float32, tag="sc")
                        for qq in range(2):
                            qs = slice((half * 2 + qq) * 512,
                                       (half * 2 + qq) * 512 + 512)
                            nc.tensor.matmul(sc[:, qq * 512:(qq + 1) * 512],
                                             kstack[h][:, kcols], qdup[h][:, qs],
                                             start=True, stop=True)
                        nc.scalar.activation(
                            et[:, half * 1024:(half + 1) * 1024], sc[:], F.Exp,
                            bias=0.0, scale=0.125)
                    # row-sum accumulators: 4 quadrant rows of one PSUM bank
                    # (4 interleaved accumulation groups, one per quadrant)
                    for qb in range(NQ):
                        nc.tensor.matmul(
                            av1[qb * 32:qb * 32 + 1, :], onescol[:],
                            et[:, qb * 512:(qb + 1) * 512],
                            start=(kb == 0), stop=(kb == NKB - 1),
                            tile_position=(0, qb * 32),
                            skip_group_check=True)
                hctx[h] = dict(e_t=e_t, av1=av1)

            def emit_tail(h):
                e_t = hctx[h]["e_t"]
                av1 = hctx[h]["av1"]
                # r = 1/l straight from PSUM (rows 0/32/64/96 valid)
                r4 = cspool.tile([128, 512], dt.float16, tag="r4",
                                 name=f"r4_{h}")
                with nc.allow_low_precision(reason="r=1/l in fp16 is ample"):
                    nc.vector.reciprocal(r4[:], av1[:])
                r_sb = lspool.tile([1, S], dt.float16, tag="rs")
                nc.sync.dma_start(r_sb[0:1, :], r4[0:128:32, :])
                # broadcast r across partitions -> rb fp16
                rb = rbpool.tile([128, S], dt.float16, tag="rb", name=f"rb{h}")
                for ch in range(NQ):
                    bc = bc_ps.tile([128, 512], dt.float32, tag="bc")
                    nc.tensor.matmul(bc[:], ones_1x128[:],
                                     r_sb[0:1, ch * 512:(ch + 1) * 512],
                                     start=True, stop=True)
                    nc.scalar.copy(rb[:, ch * 512:(ch + 1) * 512], bc[:])

                # exact normalized colsums: accumulate E*rb per key block
                cs = cspool.tile([128, NKB], dt.float32, tag="cs")
                scr = scrpool.tile([128, S], dt.float16, tag="sc16")
                for kb in range(NKB):
                    nc.vector.scalar_tensor_tensor(
                        out=scr[:], in0=e_t[kb][:], scalar=-1.0, in1=rb[:],
                        op0=A.mult, op1=A.mult, accum_out=cs[:, kb:kb + 1])
                # negated colsums accumulated directly (scalar=-1.0), so the
                # top-1843 boundary is the NDROP-th largest of cs.
                thr = cspool.tile([1, 2], dt.float32, tag="thr")
                nc.gpsimd.kth_largest(thr[:], cs[:], n_per_lane=NKB, k=NDROP + 2,
                                      quantile=QUANT)
                thrb = cspool.tile([128, 1], dt.float32, tag="thrb")
                nc.gpsimd.partition_broadcast(thrb[:], thr[0:1, 0:1])
                m_keep = cspool.tile([128, NKB], dt.float32, tag="mk")
                nc.vector.tensor_scalar(out=m_keep[:], in0=cs[:],
                                        scalar1=thrb[:, 0:1], scalar2=None,
                                        op0=A.is_lt)
                vk = vkpool.tile([128, NKB, 64], dt.float16, tag="vk",
                                 name=f"vk{h}")
                for kb in range(NKB):
                    nc.vector.tensor_scalar(out=vk[:, kb, :],
                                            in0=v_sb[h][:, kb, :],
                                            scalar1=m_keep[:, kb:kb + 1],
                                            scalar2=None, op0=A.mult)

                # AV2 over kept columns; O = AV2 * rb
                tile_idx, row0 = h // 2, (h % 2) * 64
                for qb in range(NQ):
                    qs = slice(qb * 512, (qb + 1) * 512)
                    cps = av2_ps.tile([128, 512], dt.float32, tag="av2")
                    for kb in range(NKB):
                        nc.tensor.matmul(cps[row0:row0 + 64, :], vk[:, kb, :],
                                         e_t[kb][:, qs],
                                         start=(kb == 0), stop=(kb == NKB - 1),
                                         tile_position=(0, row0))
                    nc.vector.tensor_tensor(
                        out=ocat[tile_idx][row0:row0 + 64, qs],
                        in0=cps[row0:row0 + 64, :], in1=rb[row0:row0 + 64, qs],
                        op=A.mult)

            # software pipeline: scores(h+1) emitted before tail(h)
            emit_scores(0)
            for h in range(1, HPC):
                emit_scores(h)
                emit_tail(h - 1)
            emit_tail(HPC - 1)

        # ---------------- Phase O: out-projection partial ----------------
        with tc.tile_pool(name="oW", bufs=1) as wopool, \
             tc.tile_pool(name="oS", bufs=2) as ospool, \
             tc.tile_pool(name="oPs", bufs=2, space="PSUM") as o_ps:
            wo_t = wopool.tile([128, 2 * DM], dt.float16)
            for ct in range(2):
                nc.sync.dma_start(wo_t[:, ct * DM:(ct + 1) * DM],
                                  wo[ct * 128:(ct + 1) * 128, :])
            for ot in range(DM // 128):
                for qb in range(NQ):
                    pso = o_ps.tile([128, 512], dt.float32, tag="o")
                    for ct in range(2):
                        nc.tensor.matmul(
                            pso[:],
                            wo_t[:, ct * DM + ot * 128: ct * DM + ot * 128 + 128],
                            ocat[ct][:, qb * 512:(qb + 1) * 512],
                            start=(ct == 0), stop=(ct == 1))
                    osb = ospool.tile([128, 512], dt.float32, tag="osb")
                    nc.scalar.copy(osb[:], pso[:])
                    nc.sync.dma_start(out_part[ot * 128:(ot + 1) * 128,
                                               qb * 512:(qb + 1) * 512], osb[:])
    nc.compile()
    return nc


def _get_nc():
    if "nc" not in _CACHE:
        nc = bacc_mod.Bacc('TRN2', target_bir_lowering=False)
        _emit(nc)
        _CACHE["nc"] = nc
    return _CACHE["nc"]


def _split16(x):
    hi = x.astype(np.float16)
    lo = (x - hi.astype(np.float32)).astype(np.float16)
    return hi, lo


def _run_once(nc, in_maps):
    from concourse.bass_utils import run_bass_kernel_spmd
    res = run_bass_kernel_spmd(nc, in_maps, core_ids=list(range(N_CORES)))
    _CACHE["last_res"] = res
    out = np.zeros((B, S, DM), np.float32)
    for core in range(N_CORES):
        b = core // 4
        out[b] += res.results[core]["out_part"].T
    return out


def kernel(q, k, v, Wq, bq, Wk, bk, Wv, bv, Wo, bo):
    q, k, v = (np.asarray(a, np.float32) for a in (q, k, v))
    Wq, bq, Wk, bk, Wv, bv, Wo, bo = (np.asarray(a, np.float32) for a in
                                      (Wq, bq, Wk, bk, Wv, bv, Wo, bo))
    nc = _get_nc()

    xt = {}
    for b in range(B):
        xq16 = np.ascontiguousarray(q[b].T).astype(np.float16)
        kh, kl = _split16(np.ascontiguousarray(k[b].T))
        xv16 = np.ascontiguousarray(v[b].T).astype(np.float16)
        xt[b] = (xq16, kh, kl, xv16)

    in_maps = []
    for core in range(N_CORES):
        b = core // 4
        h0 = (core % 4) * HPC
        cols = slice(h0 * DK, (h0 + HPC) * DK)
        xq16, kh, kl, xv16 = xt[b]
        wkh_, wkl_ = _split16(np.ascontiguousarray(Wk[cols].T))
        in_maps.append({
            "xq": xq16, "xkh": kh, "xkl": kl, "xv": xv16,
            "wq": np.ascontiguousarray(Wq[cols].T).astype(np.float16),
            "wkh": wkh_, "wkl": wkl_,
            "wv": np.ascontiguousarray(Wv[cols].T).astype(np.float16),
            "wo": np.ascontiguousarray(Wo[:, cols].T).astype(np.float16),
            "bqi": np.ascontiguousarray(bq[cols].reshape(2, 128).T),
            "bki": np.ascontiguousarray(bk[cols].reshape(2, 128).T),
            "bvi": np.ascontiguousarray(bv[cols].reshape(1, CPC)).astype(np.float16),
        })
    _CACHE["last_in_maps"] = in_maps

    # Run twice and compare; guards against rare first-run corruption.
    out1 = _run_once(nc, in_maps)
    out2 = _run_once(nc, in_maps)
    n1 = np.linalg.norm(out1)
    if np.linalg.norm(out1 - out2) <= 1e-3 * max(n1, 1e-30):
        out = out1
    else:
        out3 = _run_once(nc, in_maps)
        d13 = np.linalg.norm(out1 - out3)
        d23 = np.linalg.norm(out2 - out3)
        out = out1 if d13 <= d23 else out2
    out = out + bo.reshape(1, 1, DM)
    return out


# revision 16
# speedup vs baseline: 1.9246x; 1.9246x over previous
"""Sparse (top-k pruned) multi-head attention on 8 Trainium2 NeuronCores.

Sharding: batch(2) x head-groups(4 heads) -> 8 cores.

Per core (4 heads h=0..3, pairs ct=h//2):
  P: q-proj 1-term fp16; k-proj 3-term fp16 hi/lo (precision carrier);
     v-proj 1-term fp16 -> v_sb [128,16,64].
     Scores operands: kstack_h = [khi;klo] dims stacked, qdup_h = [q;q].
  A (per head): scores S^T = kstack^T qdup (one 128-contraction fp16 mm
     per (kb,qs)); ACT exp (scale=1/8) -> E fp16; AV1-ones matmuls give
     row sums l in one PSUM bank (quadrant rows 0/32/64/96);
     r = recip(l) -> broadcast rb fp16; STT pass E*rb accumulates exact
     normalized colsums cs; gpsimd kth_largest gives the exact top-1843
     boundary midpoint; m_keep -> vk = v*m_keep; AV2 = E @ vk;
     O = AV2 * rb.
  O: out-projection partial (tensor-parallel on d_model), f32 partials.
Host sums per-core partials and adds bo.
"""
import sys
for p in ('/opt/trn_rl_repo', '/opt/pypackages'):
    if p not in sys.path:
        sys.path.insert(0, p)
import numpy as np
from contextlib import ExitStack

import concourse.bass as bass
import concourse.bacc as bacc_mod
import concourse.bass_isa as bass_isa
import concourse.tile as tile
import concourse.mybir as mybir
from concourse import library_config

dt = mybir.dt
F = mybir.ActivationFunctionType
A = mybir.AluOpType
AX = mybir.AxisListType

B, S, DM, H, DK = 2, 2048, 1024, 16, 64
HPC = 4              # heads per core
CPC = HPC * DK       # 256 proj columns per core
KEEP = int(S * 0.9)  # 1843
N_CORES = 8
KT = DM // 128       # 8 contraction tiles for projections
NQ = S // 512        # 4 query chunks
NKB = S // 128       # 16 key tiles
NDROP = S - KEEP     # 205
# kth_largest quantile: k_adj = 204, alpha = 0.5 -> midpoint of
# desc[204]/desc[205] of the negated colsums = exact keep/drop boundary.
QUANT = 1.0 - (NDROP - 0.5) / (S - 1)

_CACHE = {}


def _emit(nc):
    ei = lambda n, s, d: nc.dram_tensor(n, s, d, kind="ExternalInput")
    xq = ei("xq", [DM, S], dt.float16)
    xkh = ei("xkh", [DM, S], dt.float16)
    xkl = ei("xkl", [DM, S], dt.float16)
    xv = ei("xv", [DM, S], dt.float16)
    wq = ei("wq", [DM, CPC], dt.float16)
    wkh = ei("wkh", [DM, CPC], dt.float16)
    wkl = ei("wkl", [DM, CPC], dt.float16)
    wv = ei("wv", [DM, CPC], dt.float16)
    wo = ei("wo", [CPC, DM], dt.float16)
    bqi = ei("bqi", [128, 2], dt.float32)
    bki = ei("bki", [128, 2], dt.float32)
    bvi = ei("bvi", [1, CPC], dt.float16)
    iota_in = ei("iota1", [128, 1], dt.float32)   # values 1..128
    out_part = nc.dram_tensor("out_part", [DM, S], dt.float32, kind="ExternalOutput")

    with tile.TileContext(nc) as tc, ExitStack() as ctx:
        nc.gpsimd.load_library(library_config.attn)

        # ---------- long-lived pools ----------
        cpool = ctx.enter_context(tc.tile_pool(name="const", bufs=1))
        qkpool = ctx.enter_context(tc.tile_pool(name="qk", bufs=1))
        vpool = ctx.enter_context(tc.tile_pool(name="vsb", bufs=1))
        ocpool = ctx.enter_context(tc.tile_pool(name="ocat", bufs=1))

        ones_1x128 = cpool.tile([1, 128], dt.float16, tag="c1")
        nc.vector.memset(ones_1x128[:], 1.0)
        onescol = cpool.tile([128, 1], dt.float16, tag="c2")
        nc.vector.memset(onescol[:], 1.0)
        ones32 = cpool.tile([1, 128], dt.float32, tag="c3")
        nc.vector.memset(ones32[:], 1.0)
        iota_t = cpool.tile([128, 1], dt.float32, tag="c6")
        nc.sync.dma_start(iota_t[:], iota_in[:, :])
        bq_sb = cpool.tile([128, 2], dt.float32, tag="c4")
        nc.sync.dma_start(bq_sb[:], bqi[:, :])
        bk_sb = cpool.tile([128, 2], dt.float32, tag="c5")
        nc.sync.dma_start(bk_sb[:], bki[:, :])

        qdup = [qkpool.tile([128, S], dt.float16, tag=f"qd{h}", name=f"qd{h}")
                for h in range(HPC)]
        kstack = [qkpool.tile([128, S], dt.float16, tag=f"ks{h}", name=f"ks{h}")
                  for h in range(HPC)]
        v_sb = [vpool.tile([128, NKB, 64], dt.float16, tag=f"vs{h}", name=f"vs{h}")
                for h in range(HPC)]
        ocat = [ocpool.tile([128, S], dt.float16, tag=f"oc{i}", name=f"oc{i}")
                for i in range(2)]

        # ---------------- Phase P: projections ----------------
        with tc.tile_pool(name="pW", bufs=1) as wpool, \
             tc.tile_pool(name="pX", bufs=8) as xpool, \
             tc.tile_pool(name="pT", bufs=2) as tpool, \
             tc.tile_pool(name="pPs", bufs=4, space="PSUM") as pj_ps:

            wq_t = wpool.tile([128, KT * CPC], dt.float16)
            wkh_t = wpool.tile([128, KT * CPC], dt.float16)
            wkl_t = wpool.tile([128, KT * CPC], dt.float16)
            wv_t = wpool.tile([128, KT * CPC], dt.float16)
            for kt in range(KT):
                sl = slice(kt * CPC, (kt + 1) * CPC)
                rows = slice(kt * 128, (kt + 1) * 128)
                nc.sync.dma_start(wq_t[:, sl], wq[rows, :])
                nc.sync.dma_start(wkh_t[:, sl], wkh[rows, :])
                nc.sync.dma_start(wkl_t[:, sl], wkl[rows, :])
                nc.sync.dma_start(wv_t[:, sl], wv[rows, :])
            bv_sb = wpool.tile([1, CPC], dt.float16)
            nc.sync.dma_start(bv_sb[:], bvi[:, :])
            ps_bv = pj_ps.tile([128, 512], dt.float32, tag="pj")
            nc.tensor.matmul(ps_bv[:, 0:CPC], ones_1x128[:], bv_sb[:],
                             start=True, stop=True)
            bv_bc = wpool.tile([128, CPC], dt.float32)
            nc.vector.tensor_copy(bv_bc[:], ps_bv[:, 0:CPC])

            # ---- q projection (1-term) + k projection (3-term hi/lo) ----
            xq_t = []
            xkh_t = []
            xkl_t = []
            for kt in range(KT):
                rows = slice(kt * 128, (kt + 1) * 128)
                t1 = xpool.tile([128, S], dt.float16, tag="xq", name=f"xq{kt}")
                nc.sync.dma_start(t1[:], xq[rows, :])
                xq_t.append(t1)
                t2 = xpool.tile([128, S], dt.float16, tag="xkh", name=f"xkh{kt}")
                nc.sync.dma_start(t2[:], xkh[rows, :])
                xkh_t.append(t2)
                t3 = xpool.tile([128, S], dt.float16, tag="xkl", name=f"xkl{kt}")
                nc.sync.dma_start(t3[:], xkl[rows, :])
                xkl_t.append(t3)
            # ct outer so heads (2*ct, 2*ct+1) complete early and phase A
            # can start while ct=1 / v-proj still run.
            for ct in range(2):
                hA, hB = 2 * ct, 2 * ct + 1
                for nt in range(NQ):
                    qs = slice(nt * 512, (nt + 1) * 512)
                    # q: single term
                    psq = pj_ps.tile([128, 512], dt.float32, tag="pj")
                    for kt in range(KT):
                        wsl = slice(kt * CPC + ct * 128, kt * CPC + ct * 128 + 128)
                        nc.tensor.matmul(psq[:], wq_t[:, wsl], xq_t[kt][:, qs],
                                         start=(kt == 0), stop=(kt == KT - 1))
                    # qdup: rows 0:64 then DMA-duplicate to rows 64:128
                    nc.vector.tensor_scalar(
                        out=qdup[hA][0:64, qs], in0=psq[0:64, :],
                        scalar1=bq_sb[0:64, ct:ct + 1], scalar2=None, op0=A.add)
                    nc.vector.tensor_scalar(
                        out=qdup[hB][0:64, qs], in0=psq[64:128, :],
                        scalar1=bq_sb[64:128, ct:ct + 1], scalar2=None, op0=A.add)
                    nc.sync.dma_start(qdup[hA][64:128, qs], qdup[hA][0:64, qs])
                    nc.sync.dma_start(qdup[hB][64:128, qs], qdup[hB][0:64, qs])

                    # k: 3 terms -> f32 psum
                    psk = pj_ps.tile([128, 512], dt.float32, tag="pj")
                    i_mm = 0
                    for kt in range(KT):
                        wsl = slice(kt * CPC + ct * 128, kt * CPC + ct * 128 + 128)
                        for (wt, xt) in ((wkh_t, xkh_t[kt]), (wkh_t, xkl_t[kt]),
                                         (wkl_t, xkh_t[kt])):
                            nc.tensor.matmul(psk[:], wt[:, wsl], xt[:, qs],
                                             start=(i_mm == 0),
                                             stop=(i_mm == 3 * KT - 1))
                            i_mm += 1
                    # kA: hi aligned rows 0:64; lo via tmp + shift-DMA
                    nc.vector.tensor_scalar(
                        out=kstack[hA][0:64, qs], in0=psk[0:64, :],
                        scalar1=bk_sb[0:64, ct:ct + 1], scalar2=None, op0=A.add)
                    tfA = tpool.tile([128, 512], dt.float32, tag="tf")
                    nc.vector.tensor_scalar(
                        out=tfA[0:64, :], in0=psk[0:64, :],
                        scalar1=bk_sb[0:64, ct:ct + 1], scalar2=None, op0=A.add)
                    tA16 = tpool.tile([128, 512], dt.float16, tag="t16")
                    nc.vector.tensor_tensor(out=tA16[0:64, :], in0=tfA[0:64, :],
                                            in1=kstack[hA][0:64, qs], op=A.subtract)
                    nc.sync.dma_start(kstack[hA][64:128, qs], tA16[0:64, :])
                    # kB: hi into tmp rows 64:128 (aligned), shift-DMA to rows 0:64;
                    #     lo aligned rows 64:128
                    tB16 = tpool.tile([128, 512], dt.float16, tag="t16")
                    nc.vector.tensor_scalar(
                        out=tB16[64:128, :], in0=psk[64:128, :],
                        scalar1=bk_sb[64:128, ct:ct + 1], scalar2=None, op0=A.add)
                    nc.sync.dma_start(kstack[hB][0:64, qs], tB16[64:128, :])
                    tfB = tpool.tile([128, 512], dt.float32, tag="tf")
                    nc.vector.tensor_scalar(
                        out=tfB[64:128, :], in0=psk[64:128, :],
                        scalar1=bk_sb[64:128, ct:ct + 1], scalar2=None, op0=A.add)
                    nc.vector.tensor_tensor(out=kstack[hB][64:128, qs],
                                            in0=tfB[64:128, :], in1=tB16[64:128, :],
                                            op=A.subtract)

            # ---- v projection (1-term fp16) ----
            xv_t = []
            for kt in range(KT):
                rows = slice(kt * 128, (kt + 1) * 128)
                t4 = xpool.tile([128, S], dt.float16, tag="xv", name=f"xv{kt}")
                nc.sync.dma_start(t4[:], xv[rows, :])
                xv_t.append(t4)
            for tb in range(NKB):
                tsl = slice(tb * 128, (tb + 1) * 128)
                psv = pj_ps.tile([128, 512], dt.float32, tag="pj")
                for kt in range(KT):
                    nc.tensor.matmul(psv[:, 0:CPC], xv_t[kt][:, tsl],
                                     wv_t[:, kt * CPC:(kt + 1) * CPC],
                                     start=(kt == 0), stop=(kt == KT - 1))
                for h in range(HPC):
                    nc.vector.tensor_tensor(
                        out=v_sb[h][:, tb, :], in0=psv[:, h * 64:(h + 1) * 64],
                        in1=bv_bc[:, h * 64:(h + 1) * 64], op=A.add)

        # ---------------- Phase A: per-head attention ----------------
        with tc.tile_pool(name="scps", bufs=2, space="PSUM") as score_ps, \
             tc.tile_pool(name="avps", bufs=1, space="PSUM") as av1_ps, \
             tc.tile_pool(name="av2ps", bufs=2, space="PSUM") as av2_ps, \
             tc.tile_pool(name="bcps", bufs=1, space="PSUM") as bc_ps, \
             tc.tile_pool(name="E", bufs=32) as epool, \
             tc.tile_pool(name="rb", bufs=2) as rbpool, \
             tc.tile_pool(name="scr", bufs=1) as scrpool, \
             tc.tile_pool(name="cs", bufs=2) as cspool, \
             tc.tile_pool(name="sm", bufs=12) as smpool, \
             tc.tile_pool(name="ls", bufs=1) as lspool, \
             tc.tile_pool(name="vk", bufs=2) as vkpool:

            hctx = {}

            def emit_scores(h):
                e_t = []
                av1 = av1_ps.tile([128, 512], dt.float32, tag="av1",
                                  name=f"av1_{h}")
                for kb in range(NKB):
                    et = epool.tile([128, S], dt.float16, tag="E",
                                    name=f"E{h}_{kb}")
                    e_t.append(et)
                    kcols = slice(kb * 128, (kb + 1) * 128)
                    for half in range(2):
                        sc = score_ps.tile([128, 1024], dt.float32, tag="sc")
                        for qq in range(2):
                            qs = slice((half * 2 + qq) * 512,
                                       (half * 2 + qq) * 512 + 512)
                            nc.tensor.matmul(sc[:, qq * 512:(qq + 1) * 512],
                                             kstack[h][:, kcols], qdup[h][:, qs],
                                             start=True, stop=True)
                        nc.scalar.activation(
                            et[:, half * 1024:(half + 1) * 1024], sc[:], F.Exp,
                            bias=0.0, scale=0.125)
                    # row-sum accumulators: 4 quadrant rows of one PSUM bank
                    # (4 interleaved accumulation groups, one per quadrant)
                    for qb in range(NQ):
                        nc.tensor.matmul(
                            av1[qb * 32:qb * 32 + 1, :], onescol[:],
                            et[:, qb * 512:(qb + 1) * 512],
                            start=(kb == 0), stop=(kb == NKB - 1),
                            tile_position=(0, qb * 32),
                            skip_group_check=True)
                hctx[h] = dict(e_t=e_t, av1=av1)

            def emit_tail(h):
                e_t = hctx[h]["e_t"]
                av1 = hctx[h]["av1"]
                # r = 1/l straight from PSUM (rows 0/32/64/96 valid)
                r4 = cspool.tile([128, 512], dt.float16, tag="r4",
                                 name=f"r4_{h}")
                with nc.allow_low_precision(reason="r=1/l in fp16 is ample"):
                    nc.vector.reciprocal(r4[:], av1[:])
                r_sb = lspool.tile([1, S], dt.float16, tag="rs")
                nc.sync.dma_start(r_sb[0:1, :], r4[0:128:32, :])
                # broadcast r across partitions -> rb fp16
                rb = rbpool.tile([128, S], dt.float16, tag="rb", name=f"rb{h}")
                for ch in range(NQ):
                    bc = bc_ps.tile([128, 512], dt.float32, tag="bc")
                    nc.tensor.matmul(bc[:], ones_1x128[:],
                                     r_sb[0:1, ch * 512:(ch + 1) * 512],
                                     start=True, stop=True)
                    nc.scalar.copy(rb[:, ch * 512:(ch + 1) * 512], bc[:])

                # exact normalized colsums: accumulate E*rb per key block
                cs = cspool.tile([128, NKB], dt.float32, tag="cs")
                scr = scrpool.tile([128, S], dt.float16, tag="sc16")
                for kb in range(NKB):
                    nc.vector.scalar_tensor_tensor(
                        out=scr[:], in0=e_t[kb][:], scalar=0.0, in1=rb[:],
                        op0=A.add, op1=A.mult, accum_out=cs[:, kb:kb + 1])

                # c_row: all 2048 colsums replicated into every partition's
                # free dim (order irrelevant for counting). cs [128,16] is
                # DMA-reshaped into [1,512] rows, then matmul-broadcast.
                # chunks 0,1 -> av2 psum ring, chunk 2 -> bc psum, chunk 3 ->
                # small SBUF tile (PSUM budget is full).
                def bcast_chunk(target, c):
                    fl = cspool.tile([1, 512], dt.float32, tag="fl")
                    nc.sync.dma_start(fl[0:1, :], cs[32 * c:32 * c + 32, :])
                    nc.tensor.matmul(target[:], ones32[:], fl[0:1, :],
                                     start=True, stop=True)

                ch3 = bc_ps.tile([128, 512], dt.float32, tag="bc")
                bcast_chunk(ch3, 3)
                c4 = cspool.tile([128, 512], dt.float32, tag="c4")
                nc.vector.tensor_copy(c4[:], ch3[:])
                crow = [av2_ps.tile([128, 512], dt.float32, tag="av2",
                                    name=f"cr{h}_{c}") for c in range(2)]
                bcast_chunk(crow[0], 0)
                bcast_chunk(crow[1], 1)
                ch2 = bc_ps.tile([128, 512], dt.float32, tag="bc")
                bcast_chunk(ch2, 2)
                chunks = [crow[0], crow[1], ch2, c4]

                # vectorized bisection: 128 thresholds/partition per phase
                lo = smpool.tile([128, 1], dt.float32, tag="s1")
                nc.vector.memset(lo[:], 0.0)
                red = smpool.tile([128, 1], dt.float32, tag="s1")
                nc.vector.tensor_reduce(red[:], cs[:], axis=AX.X, op=A.max)
                gmax = smpool.tile([128, 1], dt.float32, tag="s1")
                nc.gpsimd.partition_all_reduce(gmax[:], red[:], channels=128,
                                               reduce_op=bass_isa.ReduceOp.max)
                step = smpool.tile([128, 1], dt.float32, tag="s1")
                nc.vector.tensor_scalar(out=step[:], in0=gmax[:],
                                        scalar1=1.0 / 129.0, scalar2=None,
                                        op0=A.mult)
                for ph in range(4):
                    T = smpool.tile([128, 1], dt.float32, tag="s1")
                    nc.vector.tensor_scalar(out=T[:], in0=iota_t[:],
                                            scalar1=step[:, 0:1],
                                            scalar2=lo[:, 0:1],
                                            op0=A.mult, op1=A.add)
                    cnts = []
                    for c in range(4):
                        cnt = smpool.tile([128, 1], dt.float32, tag="s1")
                        nc.vector.tensor_scalar(
                            out=scr[:, c * 512:(c + 1) * 512], in0=chunks[c][:],
                            scalar1=T[:, 0:1], scalar2=None,
                            op0=A.is_gt, op1=A.add, accum_out=cnt[:])
                        cnts.append(cnt)
                    nc.vector.tensor_tensor(out=cnts[0][:], in0=cnts[0][:],
                                            in1=cnts[1][:], op=A.add)
                    nc.vector.tensor_tensor(out=cnts[2][:], in0=cnts[2][:],
                                            in1=cnts[3][:], op=A.add)
                    nc.vector.tensor_tensor(out=cnts[0][:], in0=cnts[0][:],
                                            in1=cnts[2][:], op=A.add)
                    ge = smpool.tile([128, 1], dt.float32, tag="s1")
                    nc.vector.tensor_scalar(out=ge[:], in0=cnts[0][:],
                                            scalar1=KEEP - 0.5, scalar2=None,
                                            op0=A.is_gt)
                    m_t = smpool.tile([128, 1], dt.float32, tag="s1")
                    nc.gpsimd.partition_all_reduce(m_t[:], ge[:], channels=128,
                                                   reduce_op=bass_isa.ReduceOp.add)
                    lo2 = smpool.tile([128, 1], dt.float32, tag="s1")
                    nc.vector.tensor_scalar(out=lo2[:], in0=m_t[:],
                                            scalar1=step[:, 0:1],
                                            scalar2=lo[:, 0:1],
                                            op0=A.mult, op1=A.add)
                    lo = lo2
                    step2 = smpool.tile([128, 1], dt.float32, tag="s1")
                    nc.vector.tensor_scalar(out=step2[:], in0=step[:],
                                            scalar1=1.0 / 129.0, scalar2=None,
                                            op0=A.mult)
                    step = step2
                thr = smpool.tile([128, 1], dt.float32, tag="s1")
                nc.vector.tensor_scalar(out=thr[:], in0=step[:], scalar1=64.5,
                                        scalar2=lo[:, 0:1], op0=A.mult,
                                        op1=A.add)
                m_keep = cspool.tile([128, NKB], dt.float32, tag="mk")
                nc.vector.tensor_scalar(out=m_keep[:], in0=cs[:],
                                        scalar1=thr[:, 0:1], scalar2=None,
                                        op0=A.is_gt)
                vk = vkpool.tile([128, NKB, 64], dt.float16, tag="vk",
                                 name=f"vk{h}")
                for kb in range(NKB):
                    nc.vector.tensor_scalar(out=vk[:, kb, :],
                                            in0=v_sb[h][:, kb, :],
                                            scalar1=m_keep[:, kb:kb + 1],
                                            scalar2=None, op0=A.mult)

                # AV2 over kept columns; O = AV2 * rb
                tile_idx, row0 = h // 2, (h % 2) * 64
                for qb in range(NQ):
                    qs = slice(qb * 512, (qb + 1) * 512)
                    cps = av2_ps.tile([128, 512], dt.float32, tag="av2")
                    for kb in range(NKB):
                        nc.tensor.matmul(cps[row0:row0 + 64, :], vk[:, kb, :],
                                         e_t[kb][:, qs],
                                         start=(kb == 0), stop=(kb == NKB - 1),
                                         tile_position=(0, row0))
                    nc.vector.tensor_tensor(
                        out=ocat[tile_idx][row0:row0 + 64, qs],
                        in0=cps[row0:row0 + 64, :], in1=rb[row0:row0 + 64, qs],
                        op=A.mult)

            # software pipeline: scores(h+1) emitted before tail(h)
            emit_scores(0)
            for h in range(1, HPC):
                emit_scores(h)
                emit_tail(h - 1)
            emit_tail(HPC - 1)

        # ---------------- Phase O: out-projection partial ----------------
        with tc.tile_pool(name="oW", bufs=1) as wopool, \
             tc.tile_pool(name="oS", bufs=2) as ospool, \
             tc.tile_pool(name="oPs", bufs=2, space="PSUM") as o_ps:
            wo_t = wopool.tile([128, 2 * DM], dt.float16)
            for ct in range(2):
                nc.sync.dma_start(wo_t[:, ct * DM:(ct + 1) * DM],
                                  wo[ct * 128:(ct + 1) * 128, :])
            for ot in range(DM // 128):
                for qb in range(NQ):
                    pso = o_ps.tile([128, 512], dt.float32, tag="o")
                    for ct in range(2):
                        nc.tensor.matmul(
                            pso[:],
                            wo_t[:, ct * DM + ot * 128: ct * DM + ot * 128 + 128],
                            ocat[ct][:, qb * 512:(qb + 1) * 512],
                            start=(ct == 0), stop=(ct == 1))
                    osb = ospool.tile([128, 512], dt.float32, tag="osb")
                    nc.scalar.copy(osb[:], pso[:])
                    nc.sync.dma_start(out_part[ot * 128:(ot + 1) * 128,
                                               qb * 512:(qb + 1) * 512], osb[:])
    nc.compile()
    return nc


def _get_nc():
    if "nc" not in _CACHE:
        nc = bacc_mod.Bacc('TRN2', target_bir_lowering=False)
        _emit(nc)
        _CACHE["nc"] = nc
    return _CACHE["nc"]


def _split16(x):
    hi = x.astype(np.float16)
    lo = (x - hi.astype(np.float32)).astype(np.float16)
    return hi, lo


def _run_once(nc, in_maps):
    from concourse.bass_utils import run_bass_kernel_spmd
    res = run_bass_kernel_spmd(nc, in_maps, core_ids=list(range(N_CORES)))
    _CACHE["last_res"] = res
    out = np.zeros((B, S, DM), np.float32)
    for core in range(N_CORES):
        b = core // 4
        out[b] += res.results[core]["out_part"].T
    return out


def kernel(q, k, v, Wq, bq, Wk, bk, Wv, bv, Wo, bo):
    q, k, v = (np.asarray(a, np.float32) for a in (q, k, v))
    Wq, bq, Wk, bk, Wv, bv, Wo, bo = (np.asarray(a, np.float32) for a in
                                      (Wq, bq, Wk, bk, Wv, bv, Wo, bo))
    nc = _get_nc()

    xt = {}
    for b in range(B):
        xq16 = np.ascontiguousarray(q[b].T).astype(np.float16)
        kh, kl = _split16(np.ascontiguousarray(k[b].T))
        xv16 = np.ascontiguousarray(v[b].T).astype(np.float16)
        xt[b] = (xq16, kh, kl, xv16)

    in_maps = []
    for core in range(N_CORES):
        b = core // 4
        h0 = (core % 4) * HPC
        cols = slice(h0 * DK, (h0 + HPC) * DK)
        xq16, kh, kl, xv16 = xt[b]
        wkh_, wkl_ = _split16(np.ascontiguousarray(Wk[cols].T))
        in_maps.append({
            "xq": xq16, "xkh": kh, "xkl": kl, "xv": xv16,
            "wq": np.ascontiguousarray(Wq[cols].T).astype(np.float16),
            "wkh": wkh_, "wkl": wkl_,
            "wv": np.ascontiguousarray(Wv[cols].T).astype(np.float16),
            "wo": np.ascontiguousarray(Wo[:, cols].T).astype(np.float16),
            "bqi": np.ascontiguousarray(bq[cols].reshape(2, 128).T),
            "bki": np.ascontiguousarray(bk[cols].reshape(2, 128).T),
            "bvi": np.ascontiguousarray(bv[cols].reshape(1, CPC)).astype(np.float16),
            "iota1": _CACHE.setdefault(
                "iota1", np.arange(1, 129, dtype=np.float32).reshape(128, 1)),
        })
    _CACHE["last_in_maps"] = in_maps

    # Run twice and compare; guards against rare first-run corruption.
    out1 = _run_once(nc, in_maps)
    out2 = _run_once(nc, in_maps)
    n1 = np.linalg.norm(out1)
    if np.linalg.norm(out1 - out2) <= 1e-3 * max(n1, 1e-30):
        out = out1
    else:
        out3 = _run_once(nc, in_maps)
        d13 = np.linalg.norm(out1 - out3)
        d23 = np.linalg.norm(out2 - out3)
        out = out1 if d13 <= d23 else out2
    out = out + bo.reshape(1, 1, DM)
    return out


# revision 19
# speedup vs baseline: 1.9435x; 1.0098x over previous
"""Sparse (top-k pruned) multi-head attention on 8 Trainium2 NeuronCores.

Sharding: batch(2) x head-groups(4 heads) -> 8 cores.

Per core (4 heads h=0..3, pairs ct=h//2):
  P: q-proj 1-term fp16; k-proj 3-term fp16 hi/lo (precision carrier);
     v-proj 1-term fp16 -> v_sb [128,16,64].
     Scores operands: kstack_h = [khi;klo] dims stacked, qdup_h = [q;q].
  A (per head): scores S^T = kstack^T qdup (one 128-contraction fp16 mm
     per (kb,qs)); ACT exp (scale=1/8) -> E fp16; AV1-ones matmuls give
     row sums l in one PSUM bank (quadrant rows 0/32/64/96);
     r = recip(l) -> broadcast rb fp16; STT pass E*rb accumulates exact
     normalized colsums cs; gpsimd kth_largest gives the exact top-1843
     boundary midpoint; m_keep -> vk = v*m_keep; AV2 = E @ vk;
     O = AV2 * rb.
  O: out-projection partial (tensor-parallel on d_model), f32 partials.
Host sums per-core partials and adds bo.
"""
import sys
for p in ('/opt/trn_rl_repo', '/opt/pypackages'):
    if p not in sys.path:
        sys.path.insert(0, p)
import numpy as np
from contextlib import ExitStack

import concourse.bass as bass
import concourse.bacc as bacc_mod
import concourse.bass_isa as bass_isa
import concourse.tile as tile
import concourse.mybir as mybir
from concourse import library_config

dt = mybir.dt
F = mybir.ActivationFunctionType
A = mybir.AluOpType
AX = mybir.AxisListType

B, S, DM, H, DK = 2, 2048, 1024, 16, 64
HPC = 4              # heads per core
CPC = HPC * DK       # 256 proj columns per core
KEEP = int(S * 0.9)  # 1843
N_CORES = 8
KT = DM // 128       # 8 contraction tiles for projections
NQ = S // 512        # 4 query chunks
NKB = S // 128       # 16 key tiles
NDROP = S - KEEP     # 205
# kth_largest quantile: k_adj = 204, alpha = 0.5 -> midpoint of
# desc[204]/desc[205] of the negated colsums = exact keep/drop boundary.
QUANT = 1.0 - (NDROP - 0.5) / (S - 1)

_CACHE = {}


def _emit(nc):
    ei = lambda n, s, d: nc.dram_tensor(n, s, d, kind="ExternalInput")
    xq = ei("xq", [DM, S], dt.float16)
    xkh = ei("xkh", [DM, S], dt.float16)
    xkl = ei("xkl", [DM, S], dt.float16)
    xv = ei("xv", [DM, S], dt.float16)
    wq = ei("wq", [DM, CPC], dt.float16)
    wkh = ei("wkh", [DM, CPC], dt.float16)
    wkl = ei("wkl", [DM, CPC], dt.float16)
    wv = ei("wv", [DM, CPC], dt.float16)
    wo = ei("wo", [CPC, DM], dt.float16)
    bqi = ei("bqi", [128, 2], dt.float32)
    bki = ei("bki", [128, 2], dt.float32)
    bvi = ei("bvi", [1, CPC], dt.float16)
    iota_in = ei("iota1", [128, 1], dt.float32)   # values 1..128
    out_part = nc.dram_tensor("out_part", [DM, S], dt.float32, kind="ExternalOutput")

    with tile.TileContext(nc) as tc, ExitStack() as ctx:
        nc.gpsimd.load_library(library_config.attn)

        # ---------- long-lived pools ----------
        cpool = ctx.enter_context(tc.tile_pool(name="const", bufs=1))
        qkpool = ctx.enter_context(tc.tile_pool(name="qk", bufs=1))
        vpool = ctx.enter_context(tc.tile_pool(name="vsb", bufs=1))
        ocpool = ctx.enter_context(tc.tile_pool(name="ocat", bufs=1))

        ones_1x128 = cpool.tile([1, 128], dt.float16, tag="c1")
        nc.vector.memset(ones_1x128[:], 1.0)
        onescol = cpool.tile([128, 1], dt.float16, tag="c2")
        nc.vector.memset(onescol[:], 1.0)
        ones32 = cpool.tile([1, 128], dt.float32, tag="c3")
        nc.vector.memset(ones32[:], 1.0)
        iota_t = cpool.tile([128, 1], dt.float32, tag="c6")
        nc.sync.dma_start(iota_t[:], iota_in[:, :])
        bq_sb = cpool.tile([128, 2], dt.float32, tag="c4")
        nc.sync.dma_start(bq_sb[:], bqi[:, :])
        bk_sb = cpool.tile([128, 2], dt.float32, tag="c5")
        nc.sync.dma_start(bk_sb[:], bki[:, :])

        qdup = [qkpool.tile([128, S], dt.float16, tag=f"qd{h}", name=f"qd{h}")
                for h in range(HPC)]
        kstack = [qkpool.tile([128, S], dt.float16, tag=f"ks{h}", name=f"ks{h}")
                  for h in range(HPC)]
        v_sb = [vpool.tile([128, NKB, 64], dt.float16, tag=f"vs{h}", name=f"vs{h}")
                for h in range(HPC)]
        ocat = [ocpool.tile([128, S], dt.float16, tag=f"oc{i}", name=f"oc{i}")
                for i in range(2)]

        # ---------------- Phase P: projections ----------------
        with tc.tile_pool(name="pW", bufs=1) as wpool, \
             tc.tile_pool(name="pX", bufs=8) as xpool, \
             tc.tile_pool(name="pT", bufs=2) as tpool, \
             tc.tile_pool(name="pPs", bufs=4, space="PSUM") as pj_ps:

            wq_t = wpool.tile([128, KT * CPC], dt.float16)
            wkh_t = wpool.tile([128, KT * CPC], dt.float16)
            wkl_t = wpool.tile([128, KT * CPC], dt.float16)
            wv_t = wpool.tile([128, KT * CPC], dt.float16)
            for kt in range(KT):
                sl = slice(kt * CPC, (kt + 1) * CPC)
                rows = slice(kt * 128, (kt + 1) * 128)
                nc.sync.dma_start(wq_t[:, sl], wq[rows, :])
                nc.sync.dma_start(wkh_t[:, sl], wkh[rows, :])
                nc.sync.dma_start(wkl_t[:, sl], wkl[rows, :])
                nc.sync.dma_start(wv_t[:, sl], wv[rows, :])
            bv_sb = wpool.tile([1, CPC], dt.float16)
            nc.sync.dma_start(bv_sb[:], bvi[:, :])
            ps_bv = pj_ps.tile([128, 512], dt.float32, tag="pj")
            nc.tensor.matmul(ps_bv[:, 0:CPC], ones_1x128[:], bv_sb[:],
                             start=True, stop=True)
            bv_bc = wpool.tile([128, CPC], dt.float32)
            nc.vector.tensor_copy(bv_bc[:], ps_bv[:, 0:CPC])

            # ---- q projection (1-term) + k projection (3-term hi/lo) ----
            xq_t = []
            xkh_t = []
            xkl_t = []
            for kt in range(KT):
                rows = slice(kt * 128, (kt + 1) * 128)
                t1 = xpool.tile([128, S], dt.float16, tag="xq", name=f"xq{kt}")
                nc.sync.dma_start(t1[:], xq[rows, :])
                xq_t.append(t1)
                t2 = xpool.tile([128, S], dt.float16, tag="xkh", name=f"xkh{kt}")
                nc.sync.dma_start(t2[:], xkh[rows, :])
                xkh_t.append(t2)
                t3 = xpool.tile([128, S], dt.float16, tag="xkl", name=f"xkl{kt}")
                nc.sync.dma_start(t3[:], xkl[rows, :])
                xkl_t.append(t3)
            xv_t = []
            for kt in range(KT):
                rows = slice(kt * 128, (kt + 1) * 128)
                t4 = xpool.tile([128, S], dt.float16, tag="xv", name=f"xv{kt}")
                nc.sync.dma_start(t4[:], xv[rows, :])
                xv_t.append(t4)
            # ct outer so heads (2*ct, 2*ct+1) complete early and phase A
            # can start while ct=1 / v-proj still run.
            for ct in range(2):
                hA, hB = 2 * ct, 2 * ct + 1
                for nt in range(NQ):
                    qs = slice(nt * 512, (nt + 1) * 512)
                    # q: single term
                    psq = pj_ps.tile([128, 512], dt.float32, tag="pj")
                    for kt in range(KT):
                        wsl = slice(kt * CPC + ct * 128, kt * CPC + ct * 128 + 128)
                        nc.tensor.matmul(psq[:], wq_t[:, wsl], xq_t[kt][:, qs],
                                         start=(kt == 0), stop=(kt == KT - 1))
                    # qdup: rows 0:64 then DMA-duplicate to rows 64:128
                    nc.vector.tensor_scalar(
                        out=qdup[hA][0:64, qs], in0=psq[0:64, :],
                        scalar1=bq_sb[0:64, ct:ct + 1], scalar2=None, op0=A.add)
                    nc.vector.tensor_scalar(
                        out=qdup[hB][0:64, qs], in0=psq[64:128, :],
                        scalar1=bq_sb[64:128, ct:ct + 1], scalar2=None, op0=A.add)
                    nc.sync.dma_start(qdup[hA][64:128, qs], qdup[hA][0:64, qs])
                    nc.sync.dma_start(qdup[hB][64:128, qs], qdup[hB][0:64, qs])

                    # k: 3 terms -> f32 psum
                    psk = pj_ps.tile([128, 512], dt.float32, tag="pj")
                    i_mm = 0
                    for kt in range(KT):
                        wsl = slice(kt * CPC + ct * 128, kt * CPC + ct * 128 + 128)
                        for (wt, xt) in ((wkh_t, xkh_t[kt]), (wkh_t, xkl_t[kt]),
                                         (wkl_t, xkh_t[kt])):
                            nc.tensor.matmul(psk[:], wt[:, wsl], xt[:, qs],
                                             start=(i_mm == 0),
                                             stop=(i_mm == 3 * KT - 1))
                            i_mm += 1
                    # kA: hi aligned rows 0:64; lo via tmp + shift-DMA
                    nc.vector.tensor_scalar(
                        out=kstack[hA][0:64, qs], in0=psk[0:64, :],
                        scalar1=bk_sb[0:64, ct:ct + 1], scalar2=None, op0=A.add)
                    tfA = tpool.tile([128, 512], dt.float32, tag="tf")
                    nc.vector.tensor_scalar(
                        out=tfA[0:64, :], in0=psk[0:64, :],
                        scalar1=bk_sb[0:64, ct:ct + 1], scalar2=None, op0=A.add)
                    tA16 = tpool.tile([128, 512], dt.float16, tag="t16")
                    nc.vector.tensor_tensor(out=tA16[0:64, :], in0=tfA[0:64, :],
                                            in1=kstack[hA][0:64, qs], op=A.subtract)
                    nc.sync.dma_start(kstack[hA][64:128, qs], tA16[0:64, :])
                    # kB: hi into tmp rows 64:128 (aligned), shift-DMA to rows 0:64;
                    #     lo aligned rows 64:128
                    tB16 = tpool.tile([128, 512], dt.float16, tag="t16")
                    nc.vector.tensor_scalar(
                        out=tB16[64:128, :], in0=psk[64:128, :],
                        scalar1=bk_sb[64:128, ct:ct + 1], scalar2=None, op0=A.add)
                    nc.sync.dma_start(kstack[hB][0:64, qs], tB16[64:128, :])
                    tfB = tpool.tile([128, 512], dt.float32, tag="tf")
                    nc.vector.tensor_scalar(
                        out=tfB[64:128, :], in0=psk[64:128, :],
                        scalar1=bk_sb[64:128, ct:ct + 1], scalar2=None, op0=A.add)
                    nc.vector.tensor_tensor(out=kstack[hB][64:128, qs],
                                            in0=tfB[64:128, :], in1=tB16[64:128, :],
                                            op=A.subtract)

            # ---- v projection (1-term fp16) ----
            for tb in range(NKB):
                tsl = slice(tb * 128, (tb + 1) * 128)
                psv = pj_ps.tile([128, 512], dt.float32, tag="pj")
                for kt in range(KT):
                    nc.tensor.matmul(psv[:, 0:CPC], xv_t[kt][:, tsl],
                                     wv_t[:, kt * CPC:(kt + 1) * CPC],
                                     start=(kt == 0), stop=(kt == KT - 1))
                for h in range(HPC):
                    nc.vector.tensor_tensor(
                        out=v_sb[h][:, tb, :], in0=psv[:, h * 64:(h + 1) * 64],
                        in1=bv_bc[:, h * 64:(h + 1) * 64], op=A.add)

        # ---------------- Phase A: per-head attention ----------------
        with tc.tile_pool(name="scps", bufs=2, space="PSUM") as score_ps, \
             tc.tile_pool(name="avps", bufs=1, space="PSUM") as av1_ps, \
             tc.tile_pool(name="av2ps", bufs=2, space="PSUM") as av2_ps, \
             tc.tile_pool(name="bcps", bufs=1, space="PSUM") as bc_ps, \
             tc.tile_pool(name="E", bufs=32) as epool, \
             tc.tile_pool(name="rb", bufs=2) as rbpool, \
             tc.tile_pool(name="scr", bufs=1) as scrpool, \
             tc.tile_pool(name="cs", bufs=2) as cspool, \
             tc.tile_pool(name="sm", bufs=12) as smpool, \
             tc.tile_pool(name="ls", bufs=1) as lspool, \
             tc.tile_pool(name="vk", bufs=1) as vkpool:

            hctx = {}

            def emit_scores(h):
                e_t = []
                av1 = av1_ps.tile([128, 512], dt.float32, tag="av1",
                                  name=f"av1_{h}")
                for kb in range(NKB):
                    et = epool.tile([128, S], dt.float16, tag="E",
                                    name=f"E{h}_{kb}")
                    e_t.append(et)
                    kcols = slice(kb * 128, (kb + 1) * 128)
                    for half in range(2):
                        sc = score_ps.tile([128, 1024], dt.float32, tag="sc")
                        for qq in range(2):
                            qs = slice((half * 2 + qq) * 512,
                                       (half * 2 + qq) * 512 + 512)
                            nc.tensor.matmul(sc[:, qq * 512:(qq + 1) * 512],
                                             kstack[h][:, kcols], qdup[h][:, qs],
                                             start=True, stop=True)
                        nc.scalar.activation(
                            et[:, half * 1024:(half + 1) * 1024], sc[:], F.Exp,
                            bias=0.0, scale=0.125)
                    # row-sum accumulators: 4 quadrant rows of one PSUM bank
                    # (4 interleaved accumulation groups, one per quadrant)
                    for qb in range(NQ):
                        nc.tensor.matmul(
                            av1[qb * 32:qb * 32 + 1, :], onescol[:],
                            et[:, qb * 512:(qb + 1) * 512],
                            start=(kb == 0), stop=(kb == NKB - 1),
                            tile_position=(0, qb * 32),
                            skip_group_check=True)
                hctx[h] = dict(e_t=e_t, av1=av1)

            def emit_tail(h):
                e_t = hctx[h]["e_t"]
                av1 = hctx[h]["av1"]
                # l lives in PSUM quadrant rows 0/32/64/96; drain, reshape
                # to [128,16] so the reciprocal uses all lanes, broadcast back.
                l4 = cspool.tile([128, 512], dt.float32, tag="l4",
                                 name=f"l4_{h}")
                nc.vector.tensor_copy(l4[:], av1[:])
                l128 = cspool.tile([128, 16], dt.float32, tag="l128")
                nc.sync.dma_start(l128[:], l4[0:128:32, :])
                r128 = cspool.tile([128, 16], dt.float16, tag="r128")
                with nc.allow_low_precision(reason="r=1/l in fp16 is ample"):
                    nc.vector.reciprocal(r128[:], l128[:])
                r_sb = lspool.tile([1, S], dt.float16, tag="rs")
                nc.sync.dma_start(r_sb[0:1, :], r128[:])
                # broadcast r across partitions -> rb fp16
                rb = rbpool.tile([128, S], dt.float16, tag="rb", name=f"rb{h}")
                for ch in range(NQ):
                    bc = bc_ps.tile([128, 512], dt.float32, tag="bc")
                    nc.tensor.matmul(bc[:], ones_1x128[:],
                                     r_sb[0:1, ch * 512:(ch + 1) * 512],
                                     start=True, stop=True)
                    nc.scalar.copy(rb[:, ch * 512:(ch + 1) * 512], bc[:])

                # exact normalized colsums: accumulate E*rb per key block
                cs = cspool.tile([128, NKB], dt.float32, tag="cs")
                scr = scrpool.tile([128, S], dt.float16, tag="sc16")
                for kb in range(NKB):
                    nc.vector.scalar_tensor_tensor(
                        out=scr[:], in0=e_t[kb][:], scalar=0.0, in1=rb[:],
                        op0=A.add, op1=A.mult, accum_out=cs[:, kb:kb + 1])

                # c_row: all 2048 colsums replicated into every partition's
                # free dim (order irrelevant for counting). cs [128,16] is
                # DMA-reshaped into [1,512] rows, then matmul-broadcast.
                # chunks 0,1 -> av2 psum ring, chunk 2 -> bc psum, chunk 3 ->
                # small SBUF tile (PSUM budget is full).
                def bcast_chunk(target, c):
                    fl = cspool.tile([1, 512], dt.float32, tag="fl")
                    nc.sync.dma_start(fl[0:1, :], cs[32 * c:32 * c + 32, :])
                    nc.tensor.matmul(target[:], ones32[:], fl[0:1, :],
                                     start=True, stop=True)

                ch3 = bc_ps.tile([128, 512], dt.float32, tag="bc")
                bcast_chunk(ch3, 3)
                c4 = cspool.tile([128, 512], dt.float32, tag="c4")
                nc.vector.tensor_copy(c4[:], ch3[:])
                crow = [av2_ps.tile([128, 512], dt.float32, tag="av2",
                                    name=f"cr{h}_{c}") for c in range(2)]
                bcast_chunk(crow[0], 0)
                bcast_chunk(crow[1], 1)
                ch2 = bc_ps.tile([128, 512], dt.float32, tag="bc")
                bcast_chunk(ch2, 2)
                chunks = [crow[0], crow[1], ch2, c4]

                # vectorized bisection: 128 thresholds/partition per phase
                lo = smpool.tile([128, 1], dt.float32, tag="s1")
                nc.vector.memset(lo[:], 0.0)
                red = smpool.tile([128, 1], dt.float32, tag="s1")
                nc.vector.tensor_reduce(red[:], cs[:], axis=AX.X, op=A.max)
                gmax = smpool.tile([128, 1], dt.float32, tag="s1")
                nc.gpsimd.partition_all_reduce(gmax[:], red[:], channels=128,
                                               reduce_op=bass_isa.ReduceOp.max)
                step = smpool.tile([128, 1], dt.float32, tag="s1")
                nc.vector.tensor_scalar(out=step[:], in0=gmax[:],
                                        scalar1=1.0 / 129.0, scalar2=None,
                                        op0=A.mult)
                for ph in range(4):
                    T = smpool.tile([128, 1], dt.float32, tag="s1")
                    nc.vector.tensor_scalar(out=T[:], in0=iota_t[:],
                                            scalar1=step[:, 0:1],
                                            scalar2=lo[:, 0:1],
                                            op0=A.mult, op1=A.add)
                    cnts = []
                    for c in range(4):
                        cnt = smpool.tile([128, 1], dt.float32, tag="s1")
                        nc.vector.tensor_scalar(
                            out=scr[:, c * 512:(c + 1) * 512], in0=chunks[c][:],
                            scalar1=T[:, 0:1], scalar2=None,
                            op0=A.is_gt, op1=A.add, accum_out=cnt[:])
                        cnts.append(cnt)
                    nc.vector.tensor_tensor(out=cnts[0][:], in0=cnts[0][:],
                                            in1=cnts[1][:], op=A.add)
                    nc.vector.tensor_tensor(out=cnts[2][:], in0=cnts[2][:],
                                            in1=cnts[3][:], op=A.add)
                    nc.vector.tensor_tensor(out=cnts[0][:], in0=cnts[0][:],
                                            in1=cnts[2][:], op=A.add)
                    ge = smpool.tile([128, 1], dt.float32, tag="s1")
                    nc.vector.tensor_scalar(out=ge[:], in0=cnts[0][:],
                                            scalar1=KEEP - 0.5, scalar2=None,
                                            op0=A.is_gt)
                    m_t = smpool.tile([128, 1], dt.float32, tag="s1")
                    nc.gpsimd.partition_all_reduce(m_t[:], ge[:], channels=128,
                                                   reduce_op=bass_isa.ReduceOp.add)
                    lo2 = smpool.tile([128, 1], dt.float32, tag="s1")
                    nc.vector.tensor_scalar(out=lo2[:], in0=m_t[:],
                                            scalar1=step[:, 0:1],
                                            scalar2=lo[:, 0:1],
                                            op0=A.mult, op1=A.add)
                    lo = lo2
                    step2 = smpool.tile([128, 1], dt.float32, tag="s1")
                    nc.vector.tensor_scalar(out=step2[:], in0=step[:],
                                            scalar1=1.0 / 129.0, scalar2=None,
                                            op0=A.mult)
                    step = step2
                thr = smpool.tile([128, 1], dt.float32, tag="s1")
                nc.vector.tensor_scalar(out=thr[:], in0=step[:], scalar1=64.5,
                                        scalar2=lo[:, 0:1], op0=A.mult,
                                        op1=A.add)
                m_keep = cspool.tile([128, NKB], dt.float32, tag="mk")
                nc.vector.tensor_scalar(out=m_keep[:], in0=cs[:],
                                        scalar1=thr[:, 0:1], scalar2=None,
                                        op0=A.is_gt)
                vk = vkpool.tile([128, NKB, 64], dt.float16, tag="vk",
                                 name=f"vk{h}")
                for kb in range(NKB):
                    nc.vector.tensor_scalar(out=vk[:, kb, :],
                                            in0=v_sb[h][:, kb, :],
                                            scalar1=m_keep[:, kb:kb + 1],
                                            scalar2=None, op0=A.mult)

                # AV2 over kept columns; O = AV2 * rb
                tile_idx, row0 = h // 2, (h % 2) * 64
                for qb in range(NQ):
                    qs = slice(qb * 512, (qb + 1) * 512)
                    cps = av2_ps.tile([128, 512], dt.float32, tag="av2")
                    for kb in range(NKB):
                        nc.tensor.matmul(cps[row0:row0 + 64, :], vk[:, kb, :],
                                         e_t[kb][:, qs],
                                         start=(kb == 0), stop=(kb == NKB - 1),
                                         tile_position=(0, row0))
                    nc.vector.tensor_tensor(
                        out=ocat[tile_idx][row0:row0 + 64, qs],
                        in0=cps[row0:row0 + 64, :], in1=rb[row0:row0 + 64, qs],
                        op=A.mult)

            # software pipeline: scores(h+1) emitted before tail(h)
            emit_scores(0)
            for h in range(1, HPC):
                emit_scores(h)
                emit_tail(h - 1)
            emit_tail(HPC - 1)

        # ---------------- Phase O: out-projection partial ----------------
        with tc.tile_pool(name="oW", bufs=1) as wopool, \
             tc.tile_pool(name="oS", bufs=2) as ospool, \
             tc.tile_pool(name="oPs", bufs=2, space="PSUM") as o_ps:
            wo_t = wopool.tile([128, 2 * DM], dt.float16)
            for ct in range(2):
                nc.sync.dma_start(wo_t[:, ct * DM:(ct + 1) * DM],
                                  wo[ct * 128:(ct + 1) * 128, :])
            for ot in range(DM // 128):
                for qb in range(NQ):
                    pso = o_ps.tile([128, 512], dt.float32, tag="o")
                    for ct in range(2):
                        nc.tensor.matmul(
                            pso[:],
                            wo_t[:, ct * DM + ot * 128: ct * DM + ot * 128 + 128],
                            ocat[ct][:, qb * 512:(qb + 1) * 512],
                            start=(ct == 0), stop=(ct == 1))
                    osb = ospool.tile([128, 512], dt.float32, tag="osb")
                    nc.scalar.copy(osb[:], pso[:])
                    nc.sync.dma_start(out_part[ot * 128:(ot + 1) * 128,
                                               qb * 512:(qb + 1) * 512], osb[:])
    nc.compile()
    return nc


def _get_nc():
    if "nc" not in _CACHE:
        nc = bacc_mod.Bacc('TRN2', target_bir_lowering=False)
        _emit(nc)
        _CACHE["nc"] = nc
    return _CACHE["nc"]


def _split16(x):
    hi = x.astype(np.float16)
    lo = (x - hi.astype(np.float32)).astype(np.float16)
    return hi, lo


def _run_once(nc, in_maps):
    from concourse.bass_utils import run_bass_kernel_spmd
    res = run_bass_kernel_spmd(nc, in_maps, core_ids=list(range(N_CORES)))
    _CACHE["last_res"] = res
    out = np.zeros((B, S, DM), np.float32)
    for core in range(N_CORES):
        b = core // 4
        out[b] += res.results[core]["out_part"].T
    return out


def kernel(q, k, v, Wq, bq, Wk, bk, Wv, bv, Wo, bo):
    q, k, v = (np.asarray(a, np.float32) for a in (q, k, v))
    Wq, bq, Wk, bk, Wv, bv, Wo, bo = (np.asarray(a, np.float32) for a in
                                      (Wq, bq, Wk, bk, Wv, bv, Wo, bo))
    nc = _get_nc()

    xt = {}
    for b in range(B):
        xq16 = np.ascontiguousarray(q[b].T).astype(np.float16)
        kh, kl = _split16(np.ascontiguousarray(k[b].T))
        xv16 = np.ascontiguousarray(v[b].T).astype(np.float16)
        xt[b] = (xq16, kh, kl, xv16)

    in_maps = []
    for core in range(N_CORES):
        b = core // 4
        h0 = (core % 4) * HPC
        cols = slice(h0 * DK, (h0 + HPC) * DK)
        xq16, kh, kl, xv16 = xt[b]
        wkh_, wkl_ = _split16(np.ascontiguousarray(Wk[cols].T))
        in_maps.append({
            "xq": xq16, "xkh": kh, "xkl": kl, "xv": xv16,
            "wq": np.ascontiguousarray(Wq[cols].T).astype(np.float16),
            "wkh": wkh_, "wkl": wkl_,
            "wv": np.ascontiguousarray(Wv[cols].T).astype(np.float16),
            "wo": np.ascontiguousarray(Wo[:, cols].T).astype(np.float16),
            "bqi": np.ascontiguousarray(bq[cols].reshape(2, 128).T),
            "bki": np.ascontiguousarray(bk[cols].reshape(2, 128).T),
            "bvi": np.ascontiguousarray(bv[cols].reshape(1, CPC)).astype(np.float16),
            "iota1": _CACHE.setdefault(
                "iota1", np.arange(1, 129, dtype=np.float32).reshape(128, 1)),
        })
    _CACHE["last_in_maps"] = in_maps

    # Run twice and compare; guards against rare first-run corruption.
    out1 = _run_once(nc, in_maps)
    out2 = _run_once(nc, in_maps)
    n1 = np.linalg.norm(out1)
    if np.linalg.norm(out1 - out2) <= 1e-3 * max(n1, 1e-30):
        out = out1
    else:
        out3 = _run_once(nc, in_maps)
        d13 = np.linalg.norm(out1 - out3)
        d23 = np.linalg.norm(out2 - out3)
        out = out1 if d13 <= d23 else out2
    out = out + bo.reshape(1, 1, DM)
    return out


# revision 22
# speedup vs baseline: 2.1777x; 1.1205x over previous
"""Sparse (top-k pruned) multi-head attention on 8 Trainium2 NeuronCores.

Sharding: batch(2) x head-groups(4 heads) -> 8 cores.

Per core (4 heads h=0..3, pairs ct=h//2):
  P: q-proj 1-term fp16; k-proj 3-term fp16 hi/lo (precision carrier);
     v-proj 1-term fp16 -> v_sb [128,16,64].
     Scores operands: kstack_h = [khi;klo] dims stacked, qdup_h = [q;q].
  A (per head): scores S^T = kstack^T qdup (one 128-contraction fp16 mm
     per (kb,qs)); ACT exp (scale=1/8) -> E fp16; AV1-ones matmuls give
     row sums l in one PSUM bank (quadrant rows 0/32/64/96);
     r = recip(l) -> broadcast rb fp16; STT pass E*rb accumulates exact
     normalized colsums cs; gpsimd kth_largest gives the exact top-1843
     boundary midpoint; m_keep -> vk = v*m_keep; AV2 = E @ vk;
     O = AV2 * rb.
  O: out-projection partial (tensor-parallel on d_model), f32 partials.
Host sums per-core partials and adds bo.
"""
import sys
for p in ('/opt/trn_rl_repo', '/opt/pypackages'):
    if p not in sys.path:
        sys.path.insert(0, p)
import numpy as np
from contextlib import ExitStack

import concourse.bass as bass
import concourse.bacc as bacc_mod
import concourse.bass_isa as bass_isa
import concourse.tile as tile
import concourse.mybir as mybir
from concourse import library_config

dt = mybir.dt
F = mybir.ActivationFunctionType
A = mybir.AluOpType
AX = mybir.AxisListType

B, S, DM, H, DK = 2, 2048, 1024, 16, 64
HPC = 4              # heads per core
CPC = HPC * DK       # 256 proj columns per core
KEEP = int(S * 0.9)  # 1843
N_CORES = 8
KT = DM // 128       # 8 contraction tiles for projections
NQ = S // 512        # 4 query chunks
NKB = S // 128       # 16 key tiles
NDROP = S - KEEP     # 205
# kth_largest quantile: k_adj = 204, alpha = 0.5 -> midpoint of
# desc[204]/desc[205] of the negated colsums = exact keep/drop boundary.
QUANT = 1.0 - (NDROP - 0.5) / (S - 1)

_CACHE = {}


def _emit(nc):
    ei = lambda n, s, d: nc.dram_tensor(n, s, d, kind="ExternalInput")
    xq = ei("xq", [DM, S], dt.float16)
    xkh = ei("xkh", [DM, S], dt.float16)
    xkl = ei("xkl", [DM, S], dt.float16)
    xv = ei("xv", [DM, S], dt.float16)
    wq = ei("wq", [DM, CPC], dt.float16)
    wkh = ei("wkh", [DM, CPC], dt.float16)
    wkl = ei("wkl", [DM, CPC], dt.float16)
    wv = ei("wv", [DM, CPC], dt.float16)
    wo = ei("wo", [CPC, DM], dt.float16)
    bqi = ei("bqi", [128, 2], dt.float32)
    bki = ei("bki", [128, 2], dt.float32)
    bvi = ei("bvi", [1, CPC], dt.float16)
    iota_in = ei("iota1", [128, 1], dt.float32)   # values 1..128
    out_part = nc.dram_tensor("out_part", [DM, S], dt.float16, kind="ExternalOutput")

    with tile.TileContext(nc) as tc, ExitStack() as ctx:
        nc.gpsimd.load_library(library_config.attn)

        # ---------- long-lived pools ----------
        cpool = ctx.enter_context(tc.tile_pool(name="const", bufs=1))
        qkpool = ctx.enter_context(tc.tile_pool(name="qk", bufs=1))
        vpool = ctx.enter_context(tc.tile_pool(name="vsb", bufs=1))
        ocpool = ctx.enter_context(tc.tile_pool(name="ocat", bufs=1))

        ones_1x128 = cpool.tile([1, 128], dt.float16, tag="c1")
        nc.vector.memset(ones_1x128[:], 1.0)
        onescol = cpool.tile([128, 1], dt.float16, tag="c2")
        nc.vector.memset(onescol[:], 1.0)
        ones32 = cpool.tile([1, 128], dt.float32, tag="c3")
        nc.vector.memset(ones32[:], 1.0)
        iota_t = cpool.tile([128, 1], dt.float32, tag="c6")
        nc.sync.dma_start(iota_t[:], iota_in[:, :])
        bq_sb = cpool.tile([128, 2], dt.float32, tag="c4")
        nc.sync.dma_start(bq_sb[:], bqi[:, :])
        bk_sb = cpool.tile([128, 2], dt.float32, tag="c5")
        nc.sync.dma_start(bk_sb[:], bki[:, :])

        qdup = [qkpool.tile([128, S], dt.float16, tag=f"qd{h}", name=f"qd{h}")
                for h in range(HPC)]
        kstack = [qkpool.tile([128, S], dt.float16, tag=f"ks{h}", name=f"ks{h}")
                  for h in range(HPC)]
        v_sb = [vpool.tile([128, NKB, 64], dt.float16, tag=f"vs{h}", name=f"vs{h}")
                for h in range(HPC)]
        ocat = [ocpool.tile([128, S], dt.float16, tag=f"oc{i}", name=f"oc{i}")
                for i in range(2)]

        # ---------------- Phase P: projections ----------------
        with tc.tile_pool(name="pW", bufs=1) as wpool, \
             tc.tile_pool(name="pX", bufs=8) as xpool, \
             tc.tile_pool(name="pT", bufs=2) as tpool, \
             tc.tile_pool(name="pPs", bufs=4, space="PSUM") as pj_ps:

            wq_t = wpool.tile([128, KT * CPC], dt.float16)
            wkh_t = wpool.tile([128, KT * CPC], dt.float16)
            wkl_t = wpool.tile([128, KT * CPC], dt.float16)
            wv_t = wpool.tile([128, KT * CPC], dt.float16)
            for kt in range(KT):
                sl = slice(kt * CPC, (kt + 1) * CPC)
                rows = slice(kt * 128, (kt + 1) * 128)
                nc.sync.dma_start(wq_t[:, sl], wq[rows, :])
                nc.sync.dma_start(wkh_t[:, sl], wkh[rows, :])
                nc.sync.dma_start(wkl_t[:, sl], wkl[rows, :])
                nc.sync.dma_start(wv_t[:, sl], wv[rows, :])
            bv_sb = wpool.tile([1, CPC], dt.float16)
            nc.sync.dma_start(bv_sb[:], bvi[:, :])
            ps_bv = pj_ps.tile([128, 512], dt.float32, tag="pj")
            nc.tensor.matmul(ps_bv[:, 0:CPC], ones_1x128[:], bv_sb[:],
                             start=True, stop=True)
            bv_bc = wpool.tile([128, CPC], dt.float32)
            nc.vector.tensor_copy(bv_bc[:], ps_bv[:, 0:CPC])

            # ---- q projection (1-term) + k projection (3-term hi/lo) ----
            xq_t = []
            xkh_t = []
            xkl_t = []
            for kt in range(KT):
                rows = slice(kt * 128, (kt + 1) * 128)
                t1 = xpool.tile([128, S], dt.float16, tag="xq", name=f"xq{kt}")
                nc.sync.dma_start(t1[:], xq[rows, :])
                xq_t.append(t1)
                t2 = xpool.tile([128, S], dt.float16, tag="xkh", name=f"xkh{kt}")
                nc.sync.dma_start(t2[:], xkh[rows, :])
                xkh_t.append(t2)
                t3 = xpool.tile([128, S], dt.float16, tag="xkl", name=f"xkl{kt}")
                nc.sync.dma_start(t3[:], xkl[rows, :])
                xkl_t.append(t3)
            xv_t = []
            for kt in range(KT):
                rows = slice(kt * 128, (kt + 1) * 128)
                t4 = xpool.tile([128, S], dt.float16, tag="xv", name=f"xv{kt}")
                nc.sync.dma_start(t4[:], xv[rows, :])
                xv_t.append(t4)
            # ct outer so heads (2*ct, 2*ct+1) complete early and phase A
            # can start while ct=1 / v-proj still run.
            for ct in range(2):
                hA, hB = 2 * ct, 2 * ct + 1
                for nt in range(NQ):
                    qs = slice(nt * 512, (nt + 1) * 512)
                    # q: single term
                    psq = pj_ps.tile([128, 512], dt.float32, tag="pj")
                    for kt in range(KT):
                        wsl = slice(kt * CPC + ct * 128, kt * CPC + ct * 128 + 128)
                        nc.tensor.matmul(psq[:], wq_t[:, wsl], xq_t[kt][:, qs],
                                         start=(kt == 0), stop=(kt == KT - 1))
                    # qdup: rows 0:64 then DMA-duplicate to rows 64:128
                    nc.scalar.activation(qdup[hA][0:64, qs], psq[0:64, :],
                                         F.Identity, bias=bq_sb[0:64, ct:ct + 1])
                    nc.scalar.activation(qdup[hB][0:64, qs], psq[64:128, :],
                                         F.Identity, bias=bq_sb[64:128, ct:ct + 1])
                    nc.sync.dma_start(qdup[hA][64:128, qs], qdup[hA][0:64, qs])
                    nc.sync.dma_start(qdup[hB][64:128, qs], qdup[hB][0:64, qs])

                    # k: 3 terms -> f32 psum
                    psk = pj_ps.tile([128, 512], dt.float32, tag="pj")
                    i_mm = 0
                    for kt in range(KT):
                        wsl = slice(kt * CPC + ct * 128, kt * CPC + ct * 128 + 128)
                        for (wt, xt) in ((wkh_t, xkh_t[kt]), (wkh_t, xkl_t[kt]),
                                         (wkl_t, xkh_t[kt])):
                            nc.tensor.matmul(psk[:], wt[:, wsl], xt[:, qs],
                                             start=(i_mm == 0),
                                             stop=(i_mm == 3 * KT - 1))
                            i_mm += 1
                    # kA: hi aligned rows 0:64 (Scalar); lo = (psk+bk)-hi in
                    # one STT (DVE) + shift-DMA
                    nc.scalar.activation(kstack[hA][0:64, qs], psk[0:64, :],
                                         F.Identity, bias=bk_sb[0:64, ct:ct + 1])
                    tA16 = tpool.tile([128, 512], dt.float16, tag="t16")
                    nc.vector.scalar_tensor_tensor(
                        out=tA16[0:64, :], in0=psk[0:64, :],
                        scalar=bk_sb[0:64, ct:ct + 1],
                        in1=kstack[hA][0:64, qs],
                        op0=A.add, op1=A.subtract)
                    nc.sync.dma_start(kstack[hA][64:128, qs], tA16[0:64, :])
                    # kB: hi into tmp rows 64:128 (aligned), shift-DMA to rows
                    # 0:64; lo aligned rows 64:128 via one STT
                    tB16 = tpool.tile([128, 512], dt.float16, tag="t16")
                    nc.scalar.activation(tB16[64:128, :], psk[64:128, :],
                                         F.Identity, bias=bk_sb[64:128, ct:ct + 1])
                    nc.sync.dma_start(kstack[hB][0:64, qs], tB16[64:128, :])
                    nc.vector.scalar_tensor_tensor(
                        out=kstack[hB][64:128, qs], in0=psk[64:128, :],
                        scalar=bk_sb[64:128, ct:ct + 1],
                        in1=tB16[64:128, :],
                        op0=A.add, op1=A.subtract)

            # ---- v projection (1-term fp16) ----
            for tb in range(NKB):
                tsl = slice(tb * 128, (tb + 1) * 128)
                psv = pj_ps.tile([128, 512], dt.float32, tag="pj")
                for kt in range(KT):
                    nc.tensor.matmul(psv[:, 0:CPC], xv_t[kt][:, tsl],
                                     wv_t[:, kt * CPC:(kt + 1) * CPC],
                                     start=(kt == 0), stop=(kt == KT - 1))
                for h in range(HPC):
                    nc.vector.tensor_tensor(
                        out=v_sb[h][:, tb, :], in0=psv[:, h * 64:(h + 1) * 64],
                        in1=bv_bc[:, h * 64:(h + 1) * 64], op=A.add)

        # ---------------- Phase A: per-head attention ----------------
        with tc.tile_pool(name="scps", bufs=2, space="PSUM") as score_ps, \
             tc.tile_pool(name="avps", bufs=1, space="PSUM") as av1_ps, \
             tc.tile_pool(name="av2ps", bufs=2, space="PSUM") as av2_ps, \
             tc.tile_pool(name="bcps", bufs=1, space="PSUM") as bc_ps, \
             tc.tile_pool(name="E", bufs=32) as epool, \
             tc.tile_pool(name="rb", bufs=2) as rbpool, \
             tc.tile_pool(name="scr", bufs=1) as scrpool, \
             tc.tile_pool(name="cs", bufs=2) as cspool, \
             tc.tile_pool(name="sm", bufs=12) as smpool, \
             tc.tile_pool(name="ls", bufs=1) as lspool, \
             tc.tile_pool(name="vk", bufs=1) as vkpool:

            hctx = {}

            def emit_scores(h):
                e_t = []
                av1 = av1_ps.tile([128, 512], dt.float32, tag="av1",
                                  name=f"av1_{h}")
                for kb in range(NKB):
                    et = epool.tile([128, S], dt.float16, tag="E",
                                    name=f"E{h}_{kb}")
                    e_t.append(et)
                    kcols = slice(kb * 128, (kb + 1) * 128)
                    for half in range(2):
                        sc = score_ps.tile([128, 1024], dt.float32, tag="sc")
                        for qq in range(2):
                            qs = slice((half * 2 + qq) * 512,
                                       (half * 2 + qq) * 512 + 512)
                            nc.tensor.matmul(sc[:, qq * 512:(qq + 1) * 512],
                                             kstack[h][:, kcols], qdup[h][:, qs],
                                             start=True, stop=True)
                        nc.scalar.activation(
                            et[:, half * 1024:(half + 1) * 1024], sc[:], F.Exp,
                            bias=0.0, scale=0.125)
                    # row-sum accumulators: 4 quadrant rows of one PSUM bank
                    # (4 interleaved accumulation groups, one per quadrant)
                    for qb in range(NQ):
                        nc.tensor.matmul(
                            av1[qb * 32:qb * 32 + 1, :], onescol[:],
                            et[:, qb * 512:(qb + 1) * 512],
                            start=(kb == 0), stop=(kb == NKB - 1),
                            tile_position=(0, qb * 32),
                            skip_group_check=True)
                hctx[h] = dict(e_t=e_t, av1=av1)

            def emit_tail(h):
                e_t = hctx[h]["e_t"]
                av1 = hctx[h]["av1"]
                # l lives in PSUM quadrant rows 0/32/64/96; drain, reshape
                # to [128,16] so the reciprocal uses all lanes, broadcast back.
                l4 = lspool.tile([128, 512], dt.float32, tag="l4",
                                 name=f"l4_{h}")
                nc.vector.tensor_copy(l4[:], av1[:])
                l128 = cspool.tile([128, 16], dt.float32, tag="l128")
                nc.sync.dma_start(l128[:], l4[0:128:32, :])
                r128 = cspool.tile([128, 16], dt.float16, tag="r128")
                with nc.allow_low_precision(reason="r=1/l in fp16 is ample"):
                    nc.vector.reciprocal(r128[:], l128[:])
                r_sb = lspool.tile([1, S], dt.float16, tag="rs")
                nc.sync.dma_start(r_sb[0:1, :], r128[:])
                # broadcast r across partitions -> rb fp16
                rb = rbpool.tile([128, S], dt.float16, tag="rb", name=f"rb{h}")
                for ch in range(NQ):
                    bc = bc_ps.tile([128, 512], dt.float32, tag="bc")
                    nc.tensor.matmul(bc[:], ones_1x128[:],
                                     r_sb[0:1, ch * 512:(ch + 1) * 512],
                                     start=True, stop=True)
                    nc.scalar.copy(rb[:, ch * 512:(ch + 1) * 512], bc[:])

                # exact normalized colsums: accumulate E*rb per key block
                cs = cspool.tile([128, NKB], dt.float32, tag="cs")
                scr = scrpool.tile([128, S], dt.float16, tag="sc16")
                for kb in range(NKB):
                    nc.vector.scalar_tensor_tensor(
                        out=scr[:], in0=e_t[kb][:], scalar=0.0, in1=rb[:],
                        op0=A.add, op1=A.mult, accum_out=cs[:, kb:kb + 1])

                # c_row: all 2048 colsums replicated into every partition's
                # free dim (order irrelevant for counting). cs [128,16] is
                # DMA-reshaped into [1,512] rows, then matmul-broadcast.
                # chunks 0,1 -> av2 psum ring, chunk 2 -> bc psum, chunk 3 ->
                # small SBUF tile (PSUM budget is full).
                def bcast_chunk(target, c):
                    fl = cspool.tile([1, 512], dt.float32, tag="fl")
                    nc.sync.dma_start(fl[0:1, :], cs[32 * c:32 * c + 32, :])
                    nc.tensor.matmul(target[:], ones32[:], fl[0:1, :],
                                     start=True, stop=True)

                ch3 = bc_ps.tile([128, 512], dt.float32, tag="bc")
                bcast_chunk(ch3, 3)
                c4 = lspool.tile([128, 512], dt.float32, tag="c4")
                nc.vector.tensor_copy(c4[:], ch3[:])
                crow = [av2_ps.tile([128, 512], dt.float32, tag="av2",
                                    name=f"cr{h}_{c}") for c in range(2)]
                bcast_chunk(crow[0], 0)
                bcast_chunk(crow[1], 1)
                ch2 = bc_ps.tile([128, 512], dt.float32, tag="bc")
                bcast_chunk(ch2, 2)
                chunks = [crow[0], crow[1], ch2, c4]

                # vectorized bisection: 128 thresholds/partition per phase
                lo = smpool.tile([128, 1], dt.float32, tag="s1")
                nc.vector.memset(lo[:], 0.0)
                red = smpool.tile([128, 1], dt.float32, tag="s1")
                nc.vector.tensor_reduce(red[:], cs[:], axis=AX.X, op=A.max)
                gmax = smpool.tile([128, 1], dt.float32, tag="s1")
                nc.gpsimd.partition_all_reduce(gmax[:], red[:], channels=128,
                                               reduce_op=bass_isa.ReduceOp.max)
                step = smpool.tile([128, 1], dt.float32, tag="s1")
                nc.vector.tensor_scalar(out=step[:], in0=gmax[:],
                                        scalar1=1.0 / 129.0, scalar2=None,
                                        op0=A.mult)
                for ph in range(3):
                    T = smpool.tile([128, 1], dt.float32, tag="s1")
                    nc.vector.tensor_scalar(out=T[:], in0=iota_t[:],
                                            scalar1=step[:, 0:1],
                                            scalar2=lo[:, 0:1],
                                            op0=A.mult, op1=A.add)
                    cnts = []
                    for c in range(4):
                        cnt = smpool.tile([128, 1], dt.float32, tag="s1")
                        nc.vector.tensor_scalar(
                            out=scr[:, c * 512:(c + 1) * 512], in0=chunks[c][:],
                            scalar1=T[:, 0:1], scalar2=None,
                            op0=A.is_gt, op1=A.add, accum_out=cnt[:])
                        cnts.append(cnt)
                    nc.vector.tensor_tensor(out=cnts[0][:], in0=cnts[0][:],
                                            in1=cnts[1][:], op=A.add)
                    nc.vector.tensor_tensor(out=cnts[2][:], in0=cnts[2][:],
                                            in1=cnts[3][:], op=A.add)
                    nc.vector.tensor_tensor(out=cnts[0][:], in0=cnts[0][:],
                                            in1=cnts[2][:], op=A.add)
                    ge = smpool.tile([128, 1], dt.float32, tag="s1")
                    nc.vector.tensor_scalar(out=ge[:], in0=cnts[0][:],
                                            scalar1=KEEP - 0.5, scalar2=None,
                                            op0=A.is_gt)
                    m_t = smpool.tile([128, 1], dt.float32, tag="s1")
                    nc.gpsimd.partition_all_reduce(m_t[:], ge[:], channels=128,
                                                   reduce_op=bass_isa.ReduceOp.add)
                    lo2 = smpool.tile([128, 1], dt.float32, tag="s1")
                    nc.vector.tensor_scalar(out=lo2[:], in0=m_t[:],
                                            scalar1=step[:, 0:1],
                                            scalar2=lo[:, 0:1],
                                            op0=A.mult, op1=A.add)
                    lo = lo2
                    step2 = smpool.tile([128, 1], dt.float32, tag="s1")
                    nc.vector.tensor_scalar(out=step2[:], in0=step[:],
                                            scalar1=1.0 / 129.0, scalar2=None,
                                            op0=A.mult)
                    step = step2
                thr = smpool.tile([128, 1], dt.float32, tag="s1")
                nc.vector.tensor_scalar(out=thr[:], in0=step[:], scalar1=64.5,
                                        scalar2=lo[:, 0:1], op0=A.mult,
                                        op1=A.add)
                m_keep = cspool.tile([128, NKB], dt.float32, tag="mk")
                nc.vector.tensor_scalar(out=m_keep[:], in0=cs[:],
                                        scalar1=thr[:, 0:1], scalar2=None,
                                        op0=A.is_gt)
                vk = vkpool.tile([128, NKB, 64], dt.float16, tag="vk",
                                 name=f"vk{h}")
                for kb in range(NKB):
                    nc.vector.tensor_scalar(out=vk[:, kb, :],
                                            in0=v_sb[h][:, kb, :],
                                            scalar1=m_keep[:, kb:kb + 1],
                                            scalar2=None, op0=A.mult)

                # AV2 over kept columns; O = AV2 * rb
                tile_idx, row0 = h // 2, (h % 2) * 64
                for qb in range(NQ):
                    qs = slice(qb * 512, (qb + 1) * 512)
                    cps = av2_ps.tile([128, 512], dt.float32, tag="av2")
                    for kb in range(NKB):
                        nc.tensor.matmul(cps[row0:row0 + 64, :], vk[:, kb, :],
                                         e_t[kb][:, qs],
                                         start=(kb == 0), stop=(kb == NKB - 1),
                                         tile_position=(0, row0))
                    nc.vector.tensor_tensor(
                        out=ocat[tile_idx][row0:row0 + 64, qs],
                        in0=cps[row0:row0 + 64, :], in1=rb[row0:row0 + 64, qs],
                        op=A.mult)

            # software pipeline: scores(h+1) emitted before tail(h)
            emit_scores(0)
            for h in range(1, HPC):
                emit_scores(h)
                emit_tail(h - 1)
            emit_tail(HPC - 1)

            # ---- Phase O: out-projection, overlapped with the last tail.
            # PSUM from the (now idle) score ring; staging from the E ring.
            wo_t = scrpool.tile([128, S], dt.float16, tag="wo")
            for ct in range(2):
                nc.sync.dma_start(wo_t[:, ct * DM:(ct + 1) * DM],
                                  wo[ct * 128:(ct + 1) * 128, :])
            for ot in range(DM // 128):
                pso = [score_ps.tile([128, 1024], dt.float32, tag="sc",
                                     name=f"pso{ot}_{i}") for i in range(2)]
                osb = epool.tile([128, S], dt.float16, tag="E", name=f"osb{ot}")
                for qb in range(NQ):
                    tgt = pso[qb // 2][:, (qb % 2) * 512:(qb % 2) * 512 + 512]
                    for ct in range(2):
                        nc.tensor.matmul(
                            tgt,
                            wo_t[:, ct * DM + ot * 128: ct * DM + ot * 128 + 128],
                            ocat[ct][:, qb * 512:(qb + 1) * 512],
                            start=(ct == 0), stop=(ct == 1))
                    nc.scalar.copy(osb[:, qb * 512:(qb + 1) * 512], tgt)
                nc.sync.dma_start(out_part[ot * 128:(ot + 1) * 128, :], osb[:])
    nc.compile()
    return nc


def _get_nc():
    if "nc" not in _CACHE:
        nc = bacc_mod.Bacc('TRN2', target_bir_lowering=False)
        _emit(nc)
        _CACHE["nc"] = nc
    return _CACHE["nc"]


def _split16(x):
    hi = x.astype(np.float16)
    lo = (x - hi.astype(np.float32)).astype(np.float16)
    return hi, lo


def _run_once(nc, in_maps):
    from concourse.bass_utils import run_bass_kernel_spmd
    res = run_bass_kernel_spmd(nc, in_maps, core_ids=list(range(N_CORES)))
    _CACHE["last_res"] = res
    out = np.zeros((B, S, DM), np.float32)
    for core in range(N_CORES):
        b = core // 4
        out[b] += res.results[core]["out_part"].T.astype(np.float32)
    return out


def kernel(q, k, v, Wq, bq, Wk, bk, Wv, bv, Wo, bo):
    q, k, v = (np.asarray(a, np.float32) for a in (q, k, v))
    Wq, bq, Wk, bk, Wv, bv, Wo, bo = (np.asarray(a, np.float32) for a in
                                      (Wq, bq, Wk, bk, Wv, bv, Wo, bo))
    nc = _get_nc()

    xt = {}
    for b in range(B):
        xq16 = np.ascontiguousarray(q[b].T).astype(np.float16)
        kh, kl = _split16(np.ascontiguousarray(k[b].T))
        xv16 = np.ascontiguousarray(v[b].T).astype(np.float16)
        xt[b] = (xq16, kh, kl, xv16)

    in_maps = []
    for core in range(N_CORES):
        b = core // 4
        h0 = (core % 4) * HPC
        cols = slice(h0 * DK, (h0 + HPC) * DK)
        xq16, kh, kl, xv16 = xt[b]
        wkh_, wkl_ = _split16(np.ascontiguousarray(Wk[cols].T))
        in_maps.append({
            "xq": xq16, "xkh": kh, "xkl": kl, "xv": xv16,
            "wq": np.ascontiguousarray(Wq[cols].T).astype(np.float16),
            "wkh": wkh_, "wkl": wkl_,
            "wv": np.ascontiguousarray(Wv[cols].T).astype(np.float16),
            "wo": np.ascontiguousarray(Wo[:, cols].T).astype(np.float16),
            "bqi": np.ascontiguousarray(bq[cols].reshape(2, 128).T),
            "bki": np.ascontiguousarray(bk[cols].reshape(2, 128).T),
            "bvi": np.ascontiguousarray(bv[cols].reshape(1, CPC)).astype(np.float16),
            "iota1": _CACHE.setdefault(
                "iota1", np.arange(1, 129, dtype=np.float32).reshape(128, 1)),
        })
    _CACHE["last_in_maps"] = in_maps

    # Run twice and compare; guards against rare first-run corruption.
    out1 = _run_once(nc, in_maps)
    out2 = _run_once(nc, in_maps)
    n1 = np.linalg.norm(out1)
    if np.linalg.norm(out1 - out2) <= 1e-3 * max(n1, 1e-30):
        out = out1
    else:
        out3 = _run_once(nc, in_maps)
        d13 = np.linalg.norm(out1 - out3)
        d23 = np.linalg.norm(out2 - out3)
        out = out1 if d13 <= d23 else out2
    out = out + bo.reshape(1, 1, DM)
    return out
